# revision 1
# baseline (speedup 1.0000x reference)
"""Batched tridiagonal (Thomas) solve on 8 TRN2 NeuronCores.

System per row (alpha in [0, 0.3)):
    sub a_i = alpha_{i-1}^2, diag b_i = 1 + alpha_i^3, super c_i = alpha_{i+1}^2 + 2 alpha_{i+1}
Forward elimination denominators denom_i = b_i - g_i/denom_{i-1} (g_i = a_i c_{i-1})
are computed via the linear scan d_i = g_i d_{i-1} + (b_i - 2 g_i), using
1/x ~= 2 - x near 1 (valid: diagonal dominance keeps denom in [0.93, 1.03];
measured end-to-end rel err ~1e-5).  cp/dp/u then come from first-order
recurrences executed with the hardware tensor_tensor_scan instruction.

Sharding: pure data parallel over batch rows (256 rows/core).  Within a core,
rows are split into 128-partition blocks and columns into strips with
contraction halos (forward influence decays ~0.096/step, backward ~0.74/step),
making every (block, strip) job fully independent.
"""

import sys

sys.path.insert(0, "/opt/trn_rl_repo")

import numpy as np

from concourse import bacc, mybir, tile
from concourse import bass_utils
from concourse.ap import AP as bass_AP

F32 = mybir.dt.float32
BF16 = mybir.dt.bfloat16
OP = mybir.AluOpType

B, N = 2048, 8192
NCORES = 8
RPC = B // NCORES          # rows per core
PB = 128                   # partition block (rows per job)
STRIP = 1024               # output columns per job
HALO_L = 8                 # forward-scan warmup (contraction <= 0.0964/step)
HALO_R = 32                # backward-scan warmup (contraction <= 0.739/step)


def _act_reciprocal(nc, out, in_, scale=1.0, bias=0.0):
    """ACT Reciprocal: out = 1/(scale*in). Emitted directly (the bass wrapper
    refuses Reciprocal for generic accuracy reasons; on our inputs, |d| in
    [0.93, 1.03], HW-measured max rel err is 1.2e-5)."""
    se = nc.scalar
    return se.add_instruction(
        mybir.InstActivation(
            name=nc.get_next_instruction_name(),
            func=mybir.ActivationFunctionType.Reciprocal,
            ins=[
                se.lower_ap(in_),
                mybir.ImmediateValue(dtype=mybir.dt.float32, value=bias),
                mybir.ImmediateValue(dtype=mybir.dt.float32, value=scale),
                mybir.ImmediateValue(dtype=mybir.dt.float32, value=0.0),
            ],
            outs=[se.lower_ap(out)],
        )
    )


def build_core_program(nc, rows=RPC, n=N, strip=STRIP, halo_l=HALO_L, halo_r=HALO_R,
                       bufs=6, fr_mode="pool", b_act=True, rnh_act=True,
                       mid_lag=1, back_lag=2, c_alt=0):
    alpha_d = nc.dram_tensor("alpha", [rows, n], F32, kind="ExternalInput").ap()
    fbig_d = nc.dram_tensor("fbig", [PB, n], F32, kind="ExternalInput").ap()
    out_d = nc.dram_tensor("out", [rows, n], F32, kind="ExternalOutput").ap()

    n_blocks = (rows + PB - 1) // PB
    n_strips = (n + strip - 1) // strip
    wmax = halo_l + strip + halo_r

    with tile.TileContext(nc) as tc:
        with tc.tile_pool(name="cpool", bufs=1) as cpool:
            ones = None
            if c_alt:
                ones = cpool.tile([PB, wmax + 2], F32, tag="ones", name="t_ones")
                nc.gpsimd.memset(ones[:], 1.0)
            jobs = []
            for blk in range(n_blocks):
                for si in range(n_strips):
                    jobs.append((blk * PB, si * strip))

            def front(pool, r0, s, jidx=0):
                """DMA + coefficient prep, through g and w."""
                # uniform domain width: edge strips extend their halo inward,
                # so pad columns sit at fixed offsets and slots stay zeroed
                # after their first use.
                w = min(n, wmax)
                dom_lo = max(0, min(s - halo_l, n - w))
                dom_hi = dom_lo + w
                j = {
                    "w": w, "oo": s - dom_lo, "r0": r0, "s": s,
                    "dom_lo": dom_lo, "dom_hi": dom_hi,
                    # padded buffers: col 0 / col w+1 are zero pads for the
                    # shifted reads g_k = A2[k-1]*C[k], ncp_k = -C[k+1]*r_k.
                    "at": pool.tile([PB, wmax + 2], F32, tag="alpha", name="t_alpha"),
                    "a2h": pool.tile([PB, wmax + 2], BF16, tag="a2h", name="t_a2h"),
                    "ch": pool.tile([PB, wmax + 2], BF16, tag="ch", name="t_ch"),
                    "gt": pool.tile([PB, wmax], BF16, tag="g", name="t_g"),
                    "a3t": pool.tile([PB, wmax], F32, tag="a3", name="t_a3"),
                    "bt": pool.tile([PB, wmax + 2], F32, tag="b", name="t_b"),
                    "wt": pool.tile([PB, wmax], F32, tag="w", name="t_w"),
                    "rnh": pool.tile([PB, wmax], BF16, tag="rn", name="t_rn"),
                    "dp": pool.tile([PB, wmax], F32, tag="dp", name="t_dp"),
                    "fbj": pool.tile([PB, wmax], F32, tag="fbj", name="t_fbj"),
                }
                at, a2h, ch = j["at"], j["a2h"], j["ch"]
                nc.gpsimd.memset(at[:, 0:1], 0.0)
                nc.gpsimd.memset(at[:, w + 1 : w + 2], 0.0)
                nc.sync.dma_start(
                    out=at[:, 1 : w + 1], in_=alpha_d[r0 : r0 + PB, dom_lo:dom_hi]
                )
                nc.sync.dma_start(
                    out=j["fbj"][:, 0:w], in_=fbig_d[:, dom_lo:dom_hi]
                )
                # A2 (bf16), S = (alpha+1)^2  (ACT)
                nc.scalar.square(a2h[:, 0 : w + 2], at[:, 0 : w + 2])
                st = j["bt"]  # S staged in b's buffer
                nc.scalar.activation(
                    st[:, 0 : w + 2], at[:, 0 : w + 2],
                    mybir.ActivationFunctionType.Square, bias=1.0, scale=1.0,
                )
                # C = S - 1 = 2 alpha + alpha^2  (bf16; alternate jobs on Pool)
                if c_alt and jidx % 2 == 0:
                    nc.gpsimd.tensor_tensor(
                        out=ch[:, 0 : w + 2], in0=st[:, 0 : w + 2],
                        in1=ones[:, 0 : w + 2], op=OP.subtract,
                    )
                else:
                    nc.vector.tensor_scalar(
                        out=ch[:, 0 : w + 2], in0=st[:, 0 : w + 2], scalar1=-1.0,
                        scalar2=None, op0=OP.add,
                    )
                # A3 = alpha * A2 (Pool, mixed f32 x bf16), b = A3 + 1
                nc.gpsimd.tensor_tensor(
                    out=j["a3t"][:, 0:w], in0=at[:, 1 : w + 1],
                    in1=a2h[:, 1 : w + 1], op=OP.mult,
                )
                # g_k = A2[k-1] * C[k]  (bf16 2x)
                nc.vector.tensor_tensor(
                    out=j["gt"][:, 0:w], in0=a2h[:, 0:w], in1=ch[:, 1 : w + 1],
                    op=OP.mult,
                )
                return j

            def mid(j):
                """bm2, z-scan (z = d - 2), then rn = 1/(-z-2) = -1/d on ACT."""
                w = j["w"]
                # bm2 = b - 2 = A3 - 1: emitted here (not in front) so it does
                # not head-of-line block ACT behind the Pool A3 dependency.
                if b_act:
                    nc.scalar.activation(
                        j["bt"][:, 0:w], j["a3t"][:, 0:w],
                        mybir.ActivationFunctionType.Copy, bias=-1.0, scale=1.0,
                    )
                else:
                    nc.vector.tensor_scalar(
                        out=j["bt"][:, 0:w], in0=j["a3t"][:, 0:w], scalar1=-1.0,
                        scalar2=None, op0=OP.add,
                    )
                zt = j["a3t"]  # A3 dead after bm2
                nc.vector.tensor_tensor_scan(
                    out=zt[:, 0:w], data0=j["gt"][:, 0:w], data1=j["bt"][:, 0:w],
                    initial=0.0, op0=OP.mult, op1=OP.add,
                )
                rn = j["bt"]  # bm2 dead after z-scan; rn = -1/d (f32)
                _act_reciprocal(nc, rn[:, 0:w], zt[:, 0:w], scale=-1.0, bias=-2.0)
                if rnh_act:
                    nc.scalar.mul(j["rnh"][:, 0:w], rn[:, 0:w], 1.0)
                else:
                    nc.vector.tensor_scalar(
                        out=j["rnh"][:, 0:w], in0=rn[:, 0:w], scalar1=1.0,
                        scalar2=None, op0=OP.mult,
                    )

            def back(j):
                """ar', fr', dp-scan, ncp, u-scan, output DMA."""
                w, r0, s = j["w"], j["r0"], j["s"]
                at, a2h, ch, gt, rn = j["at"], j["a2h"], j["ch"], j["gt"], j["bt"]
                # ar'_k = A2[k-1] * rn_k  (bf16 2x, into gt; g dead)
                nc.vector.tensor_tensor(
                    out=gt[:, 0:w], in0=a2h[:, 0:w], in1=j["rnh"][:, 0:w],
                    op=OP.mult,
                )
                # fr'_k = f_k * rn_k = -f_k r_k  (into wt; w dead)
                fr = j["wt"]
                eng = nc.gpsimd if fr_mode == "pool" else nc.vector
                eng.tensor_tensor(
                    out=fr[:, 0:w], in0=j["fbj"][:, 0:w],
                    in1=rn[:, 0:w], op=OP.mult,
                )
                # dp-scan: dp_k = ar'_k * dp_{k-1} - fr'_k  (dp positive)
                nc.vector.tensor_tensor_scan(
                    out=j["dp"][:, 0:w], data0=gt[:, 0:w], data1=fr[:, 0:w],
                    initial=0.0, op0=OP.mult, op1=OP.subtract,
                )
                # ncp_k = C[k+1] * rn_k  (bf16 2x, into a2h; dead after ar')
                ncp = a2h
                nc.vector.tensor_tensor(
                    out=ncp[:, 0:w], in0=ch[:, 2 : w + 2], in1=j["rnh"][:, 0:w],
                    op=OP.mult,
                )
                # u-scan (backward): u_k = ncp_k * u_{k+1} + dp_k  (into wt)
                ut = j["wt"]
                nc.vector.tensor_tensor_scan(
                    out=ut[:, 0:w][:, ::-1],
                    data0=ncp[:, 0:w][:, ::-1],
                    data1=j["dp"][:, 0:w][:, ::-1],
                    initial=0.0, op0=OP.mult, op1=OP.add,
                )
                out_hi = min(n, s + strip)
                nc.sync.dma_start(
                    out=out_d[r0 : r0 + PB, s:out_hi],
                    in_=ut[:, j["oo"] : j["oo"] + (out_hi - s)],
                )

            # software-pipelined emission: F(k) | M(k-mid_lag) | B(k-back_lag)
            with tc.tile_pool(name="jobs", bufs=bufs) as pool:
                live = []
                for jidx, (r0, s) in enumerate(jobs):
                    live.append(front(pool, r0, s, jidx))
                    if len(live) > mid_lag:
                        mid(live[-1 - mid_lag])
                    if len(live) > back_lag:
                        back(live[-1 - back_lag])
                nj = len(live)
                for k in range(nj - mid_lag, nj):
                    if k >= 0:
                        mid(live[k])
                for k in range(nj - back_lag, nj):
                    if k >= 0:
                        back(live[k])
    return nc


_cached = None


def _get_program():
    global _cached
    if _cached is None:
        nc = bacc.Bacc("TRN2", target_bir_lowering=False, debug=False)
        build_core_program(nc)
        nc.compile()
        _cached = nc
    return _cached


def kernel(alpha: np.ndarray, f: np.ndarray) -> np.ndarray:
    alpha = np.ascontiguousarray(alpha, dtype=np.float32)
    f = np.ascontiguousarray(f, dtype=np.float32).reshape(1, N)
    nc = _get_program()
    fbig = np.ascontiguousarray(np.broadcast_to(f, (PB, N)))
    in_maps = [
        {"alpha": alpha[c * RPC : (c + 1) * RPC], "fbig": fbig}
        for c in range(NCORES)
    ]
    res = bass_utils.run_bass_kernel_spmd(nc, in_maps, core_ids=list(range(NCORES)))
    return np.concatenate([r["out"] for r in res.results], axis=0)


if __name__ == "__main__":
    rng = np.random.default_rng(0)
    a = (0.3 * rng.random((B, N))).astype(np.float32)
    fv = rng.standard_normal(N).astype(np.float32)
    u = kernel(a, fv)
    print(u.shape, u.dtype, np.abs(u).max())



# revision 11
# speedup vs baseline: 1.1282x; 1.1282x over previous
"""Batched tridiagonal (Thomas) solve on 8 TRN2 NeuronCores.

System per row (alpha in [0, 0.3)):
    sub a_i = alpha_{i-1}^2, diag b_i = 1 + alpha_i^3,
    super c_i = CS_{i+1},  CS_j = alpha_j^2 + 2 alpha_j

Forward elimination is contraction-dominated (|g| <= 0.097, |q| <= 0.11
per step), so the two forward recurrences are replaced by 2-term Neumann
expansions (numerically validated: end-to-end rel err 5.3e-3 vs 2e-2):
    nr_i ~= m3_i + g_i * m3_{i-1}          (nr ~= -1/denom; 1/x ~= 2-x)
    w_i  ~= f_i + (q*f)_{i-1}              (dp numerator)
Only the backward substitution (decay 0.77/step) runs as a real
tensor_tensor_scan:  y_i = t_{i+1}*y_{i+1} - w_i,  u = nr*y.

Engine split per (128-row, strip) job, all bf16:
  ACT : A2 = a^2, S = (a+1)^2, Q = p*(a+h)^2, C = Copy(S-1)
        (p,h,r = minimax fit of a^3 on [0,0.3), |err| <= 8.5e-4)
  DVE : m3 = Q+(r-1) [TS], products g/t1g/q/t1w/t [bf16 2x TT],
        y-scan, part of u
  Pool: g or u products (balance), SWDGE issue of the CCE adds
  DMA : alpha in, u out, shared-f broadcast, and the two Neumann "+"
        via accum-add DMAs (dst += in) on the otherwise idle DMA fleet.

Sharding: pure data parallel over batch rows (256 rows/core = 2 blocks
of 128 partitions); columns split into strips with contraction halos so
every job is independent. f is shared: one bf16 [128, 8192] broadcast
load per core. Host does dtype casts and the final fp32 cast.
"""

import sys

sys.path.insert(0, "/opt/trn_rl_repo")

import numpy as np
from ml_dtypes import bfloat16

from concourse import bacc, mybir, tile
from concourse import bass_utils

F32 = mybir.dt.float32
BF16 = mybir.dt.bfloat16
OP = mybir.AluOpType
ACT = mybir.ActivationFunctionType

B, N = 2048, 8192
NCORES = 8
RPC = B // NCORES          # rows per core
PB = 128                   # partition block (rows per job)
HALO_L = 8                 # forward warmup (contraction <= 0.11/step)
HALO_R = 32                # backward-scan warmup (contraction <= 0.77/step)

# minimax fit alpha^3 ~= P3*(alpha+H3)^2 + R3 on [0, 0.3), max err 8.44e-4
P3 = 0.45
H3 = -0.05625
R3 = -0.00058007812
SQP = float(np.sqrt(P3))            # Q = Square(SQP*alpha + SQP*H3)
SQPH = float(np.float32(SQP * H3))

DEFAULT_STRIPS = (512, 1536, 1536, 1536, 1536, 1024, 512)


def build_core_program(nc, rows=RPC, n=N, strips=DEFAULT_STRIPS,
                       halo_l=HALO_L, halo_r=HALO_R, bufs=7,
                       eng_g="dve", eng_q="dve", eng_t="dve",
                       eng_u="pool",
                       nr_mode="2t", w_mode="2t",
                       c_mode="act", m3_mode="dve",
                       lags=(1, 2, 3, 3, 4, 5), fb_chunks=4, lat_edge=(2, 4)):
    assert sum(strips) == n
    alpha_d = nc.dram_tensor("alpha", [rows, n], BF16, kind="ExternalInput").ap()
    fb_d = nc.dram_tensor("fb", [PB, n], BF16, kind="ExternalInput").ap()
    out_d = nc.dram_tensor("out", [rows, n], BF16, kind="ExternalOutput").ap()

    # bias const AP for the Q-square activation
    tb = nc.alloc_sbuf_tensor("const-q-bias", [128, 1], F32)
    nc.gpsimd.memset(tb.ap(), SQPH)
    nc.const_aps.aps[(F32, SQPH)] = tb.ap()

    n_blocks = (rows + PB - 1) // PB
    wmax = halo_l + max(strips) + halo_r

    def product(eng, out, in0, in1):
        e = nc.vector if eng == "dve" else nc.gpsimd
        e.tensor_tensor(out=out, in0=in0, in1=in1, op=OP.mult)

    def pick(eng, jidx, njobs):
        if isinstance(eng, str):
            return eng
        if isinstance(eng, (list, tuple)):
            mode, k = eng
            if mode == "head":
                return "dve" if jidx < k else "pool"
            if mode == "tail":
                return "dve" if jidx >= njobs - k else "pool"
            raise ValueError(eng)
        k = int(round(eng * njobs))
        return "pool" if jidx < k else "dve"

    with tile.TileContext(nc) as tc:
        with tc.tile_pool(name="fixed", bufs=1) as fixed:
            fb = fixed.tile([PB, n], BF16, tag="fb", name="t_fb")
            fb_pieces = [(ci * n // fb_chunks, (ci + 1) * n // fb_chunks)
                         for ci in range(fb_chunks)]

            perblk = []
            for blk in range(n_blocks):
                order = strips if blk % 2 == 0 else strips[::-1]
                pos = 0
                row = []
                for ssz in order:
                    row.append((blk * PB, pos, ssz))
                    pos += ssz
                perblk.append(row)
            jobs = [j for pair in zip(*perblk) for j in pair]

            def front(pool, r0, s, ssz, jidx, njobs):
                """alpha DMA, ACT squares, C, m3, g."""
                w = halo_l + ssz + halo_r
                dom_lo = max(0, min(s - halo_l, n - w))
                j = {
                    "r0": r0, "s": s, "oo": s - dom_lo, "w": w, "ssz": ssz,
                    "jidx": jidx, "njobs": njobs,
                    # padded tiles: reserved zero cols for shifted reads
                    "at": pool.tile([PB, wmax + 2], BF16, tag="at", name="t_at"),
                    "a2": pool.tile([PB, wmax + 2], BF16, tag="a2", name="t_a2"),
                    "ct": pool.tile([PB, wmax + 2], BF16, tag="ct", name="t_ct"),
                    "qt": pool.tile([PB, wmax + 2], BF16, tag="qt", name="t_qt"),
                    "gt": pool.tile([PB, wmax + 2], BF16, tag="gt", name="t_gt"),
                    "nr": pool.tile([PB, wmax], BF16, tag="nr", name="t_nr"),
                    "tt": pool.tile([PB, wmax + 2], BF16, tag="tt", name="t_tt"),
                }
                at, a2, ct, qt = j["at"], j["a2"], j["ct"], j["qt"]
                nc.sync.dma_start(out=at[:, 0:w],
                                  in_=alpha_d[r0:r0 + PB, dom_lo:dom_lo + w])
                nc.gpsimd.memset(a2[:, 0:1], 0.0)
                nc.gpsimd.memset(ct[:, w + 1:w + 2], 0.0)
                nc.gpsimd.memset(qt[:, 0:1], 0.0)
                nc.scalar.activation(a2[:, 1:w + 1], at[:, 0:w], ACT.Square,
                                     bias=0.0, scale=1.0)
                nc.scalar.activation(qt[:, 1:w + 1], at[:, 0:w], ACT.Square,
                                     bias=SQPH, scale=SQP)
                if c_mode == "act":
                    st = j["tt"]  # stage S in tt (dead until t)
                    nc.scalar.activation(st[:, 1:w + 1], at[:, 0:w], ACT.Square,
                                         bias=1.0, scale=1.0)
                    nc.scalar.activation(ct[:, 1:w + 1], st[:, 1:w + 1],
                                         ACT.Copy, bias=-1.0, scale=1.0)
                else:
                    nc.scalar.activation(ct[:, 1:w + 1], at[:, 0:w], ACT.Square,
                                         bias=1.0, scale=1.0)
                    nc.vector.tensor_scalar(out=ct[:, 1:w + 1], in0=ct[:, 1:w + 1],
                                            scalar1=-1.0, scalar2=None, op0=OP.add)
                if m3_mode == "act":
                    nc.scalar.activation(qt[:, 1:w + 1], qt[:, 1:w + 1], ACT.Copy,
                                         bias=R3 - 1.0, scale=1.0)
                nc.gpsimd.memset(j["gt"][:, 0:1], 0.0)
                nc.gpsimd.memset(j["at"][:, 0:1], 0.0)
                return j

            def st_prep(j):
                """m3 = Q+(r-1) [DVE TS] and g = A2[k-1]*C."""
                w, a2, ct, qt = j["w"], j["a2"], j["ct"], j["qt"]
                if m3_mode != "act":
                    nc.vector.tensor_scalar(out=qt[:, 1:w + 1], in0=qt[:, 1:w + 1],
                                            scalar1=R3 - 1.0, scalar2=None,
                                            op0=OP.add)
                eg = ("dve" if (j["jidx"] < 2 or j["jidx"] >= j["njobs"] - 4)
                      else pick(eng_g, j["jidx"], j["njobs"]))
                product(eg, j["gt"][:, 1:w + 1],
                        a2[:, 0:w], ct[:, 1:w + 1])

            def is_edge(j):
                return (j["jidx"] < lat_edge[0]
                        or j["jidx"] >= j["njobs"] - lat_edge[1])

            def st_nr(j):
                """nr = m3 + g*m3[-1] (2t) or forward scan."""
                w = j["w"]
                if nr_mode == "2t" and not is_edge(j):
                    nc.vector.tensor_tensor(out=j["nr"][:, 0:w],
                                            in0=j["gt"][:, 1:w + 1],
                                            in1=j["qt"][:, 0:w], op=OP.mult)
                    nc.gpsimd.dma_start(out=j["nr"][:, 0:w],
                                        in_=j["qt"][:, 1:w + 1], accum_op=OP.add)
                else:
                    nc.vector.tensor_tensor_scan(
                        out=j["nr"][:, 0:w], data0=j["gt"][:, 1:w + 1],
                        data1=j["qt"][:, 1:w + 1],
                        initial=0.0, op0=OP.mult, op1=OP.add,
                    )

            def st_q(j):
                """q = A2*nr into gt (g dead); t = C*nr into tt."""
                w = j["w"]
                product(pick(eng_q, j["jidx"], j["njobs"]), j["gt"][:, 1:w + 1],
                        j["a2"][:, 1:w + 1], j["nr"][:, 0:w])
                product(pick(eng_t, j["jidx"], j["njobs"]), j["tt"][:, 1:w + 1],
                        j["ct"][:, 1:w + 1], j["nr"][:, 0:w])

            def st_w(j):
                """w = f + (q*f)[-1] (2t) into at (alpha dead), or scan."""
                w = j["w"]
                dom_lo = j["s"] - j["oo"]
                fbs = fb[:, dom_lo:dom_lo + w]
                if w_mode == "2t" and not is_edge(j):
                    nc.vector.tensor_tensor(out=j["at"][:, 1:w + 1],
                                            in0=j["gt"][:, 1:w + 1],
                                            in1=fbs, op=OP.mult)
                    nc.gpsimd.dma_start(out=j["at"][:, 0:w], in_=fbs,
                                        accum_op=OP.add)
                else:
                    nc.vector.tensor_tensor_scan(
                        out=j["at"][:, 0:w], data0=j["gt"][:, 0:w], data1=fbs,
                        initial=0.0, op0=OP.mult, op1=OP.add,
                    )

            def st_y(j):
                """backward scan: y_i = t_{i+1}*y_{i+1} - w_i, into qt."""
                w = j["w"]
                nc.vector.tensor_tensor_scan(
                    out=j["qt"][:, 0:w][:, ::-1],
                    data0=j["tt"][:, 2:w + 2][:, ::-1],
                    data1=j["at"][:, 0:w][:, ::-1],
                    initial=0.0, op0=OP.mult, op1=OP.subtract,
                )

            def st_u(j):
                """u = nr*y into ct (C dead), DMA out."""
                oo, s, r0, m = j["oo"], j["s"], j["r0"], j["ssz"]
                ut = j["ct"]
                eng = "dve" if is_edge(j) else pick(eng_u, j["jidx"], j["njobs"])
                product(eng, ut[:, 0:m],
                        j["nr"][:, oo:oo + m], j["qt"][:, oo:oo + m])
                nc.sync.dma_start(out=out_d[r0:r0 + PB, s:s + m], in_=ut[:, 0:m])

            stages = [st_prep, st_nr, st_q, st_w, st_y, st_u]
            with tc.tile_pool(name="jobs", bufs=bufs) as pool:
                live = []
                nj = len(jobs)
                pieces = list(fb_pieces)
                for k in range(nj + max(lags)):
                    if k < nj:
                        r0, s, ssz = jobs[k]
                        live.append(front(pool, r0, s, ssz, k, nj))
                    if pieces:
                        lo, hi = pieces.pop(0)
                        nc.sync.dma_start(out=fb[:, lo:hi], in_=fb_d[:, lo:hi])
                    for fn, lag in zip(stages, lags):
                        i = k - lag
                        if 0 <= i < nj:
                            fn(live[i])
    return nc


_cached = None


def _get_program():
    global _cached
    if _cached is None:
        nc = bacc.Bacc("TRN2", target_bir_lowering=False, debug=False)
        build_core_program(nc)
        nc.compile()
        _cached = nc
    return _cached


def _in_maps(alpha, f):
    alpha16 = np.ascontiguousarray(alpha.astype(bfloat16))
    fb = np.ascontiguousarray(
        np.broadcast_to(f.astype(bfloat16).reshape(1, N), (PB, N))
    )
    return [
        {"alpha": alpha16[c * RPC:(c + 1) * RPC], "fb": fb}
        for c in range(NCORES)
    ]


def kernel(alpha: np.ndarray, f: np.ndarray) -> np.ndarray:
    alpha = np.ascontiguousarray(alpha, dtype=np.float32)
    f = np.ascontiguousarray(f, dtype=np.float32)
    nc = _get_program()
    res = bass_utils.run_bass_kernel_spmd(nc, _in_maps(alpha, f),
                                          core_ids=list(range(NCORES)))
    out = np.concatenate([r["out"] for r in res.results], axis=0)
    return out.astype(np.float32)


if __name__ == "__main__":
    rng = np.random.default_rng(0)
    a = (0.3 * rng.random((B, N))).astype(np.float32)
    fv = rng.standard_normal(N).astype(np.float32)
    u = kernel(a, fv)
    print(u.shape, u.dtype, np.abs(u).max())


# revision 25
# speedup vs baseline: 1.3171x; 1.1674x over previous
"""Batched tridiagonal (Thomas) solve on 8 TRN2 NeuronCores.

System per row (alpha in [0, 0.3)):
    sub a_i = alpha_{i-1}^2, diag b_i = 1 + alpha_i^3,
    super c_i = CS_{i+1},  CS_j = alpha_j^2 + 2 alpha_j

Forward elimination is contraction-dominated (|g| <= 0.097, |q| <= 0.11
per step), so both forward recurrences collapse to closed forms
(numerically validated: end-to-end rel err ~7e-3 vs the 2e-2 budget):
    nr_i ~= m3_i - g_i                     (nr ~= -1/denom; 1/x ~= 2-x,
                                            m3 = b-2 via minimax-linear a^3)
    w_i  ~= f_i + (q*f)_{i-1}              (dp numerator, 2-term Neumann)
Only the backward substitution (decay 0.77/step) runs as a real
tensor_tensor_scan:  y_i = t_{i+1}*y_{i+1} - w_i,  u = nr*y.

Engine split per (128-row, strip) job, all bf16:
  ACT : A2 = a^2, S = (a+1)^2, C = Copy(S-1)
  DVE : m3 = L1*a+(L0-1) [TS], nr = m3-g [TT], products g/q/t1w/t
        [bf16 2x TT], the backward y-scan, edge-job forward scans
  Pool: u product, a column-split share of g, SWDGE issue of the w add
  DMA : alpha in, u out, shared-f broadcast, and the w-assembly "+f"
        via an accum-add DMA (dst += in) on the otherwise idle DMA fleet.
The first/last jobs ("edge") use true forward scans and DVE-only paths
to minimize pipeline fill/drain latency; interior jobs use the
throughput path above, software-pipelined via staged lags.

Sharding: pure data parallel over batch rows (256 rows/core = 2 blocks
of 128 partitions); columns split into strips with contraction halos so
every job is independent. f is shared: one bf16 [128, 8192] broadcast
load per core. Host does dtype casts and the final fp32 cast.
"""

import sys

sys.path.insert(0, "/opt/trn_rl_repo")

import numpy as np
from ml_dtypes import bfloat16

from concourse import bacc, mybir, tile
from concourse import bass_utils

F32 = mybir.dt.float32
BF16 = mybir.dt.bfloat16
OP = mybir.AluOpType
ACT = mybir.ActivationFunctionType

B, N = 2048, 8192
NCORES = 8
RPC = B // NCORES          # rows per core
PB = 128                   # partition block (rows per job)
HALO_L = 8                 # forward warmup (contraction <= 0.11/step)
HALO_R = 32                # backward-scan warmup (contraction <= 0.77/step)

# minimax fit alpha^3 ~= P3*(alpha+H3)^2 + R3 on [0, 0.3), max err 8.44e-4
P3 = 0.45
H3 = -0.05625
R3 = -0.00058007812
SQP = float(np.sqrt(P3))            # Q = Square(SQP*alpha + SQP*H3)
SQPH = float(np.float32(SQP * H3))

DEFAULT_STRIPS = (512, 1536, 1536, 1536, 1536, 1536)


def build_core_program(nc, rows=RPC, n=N, strips=DEFAULT_STRIPS,
                       halo_l=HALO_L, halo_r=HALO_R, bufs=8,
                       eng_g=("split", 0.25), eng_q="dve", eng_t="dve",
                       eng_u="pool",
                       nr_mode="sub", w_mode="2t",
                       c_mode="act", m3_mode="lin",
                       lags=(1, 1, 3, 3, 4, 5), fb_chunks=4, lat_edge=(1, 2)):
    assert sum(strips) == n
    alpha_d = nc.dram_tensor("alpha", [rows, n], BF16, kind="ExternalInput").ap()
    fb_d = nc.dram_tensor("fb", [PB, n], BF16, kind="ExternalInput").ap()
    out_d = nc.dram_tensor("out", [rows, n], BF16, kind="ExternalOutput").ap()

    if m3_mode not in ("lin", "actlin"):
        # bias const AP for the Q-square activation
        tb = nc.alloc_sbuf_tensor("const-q-bias", [128, 1], F32)
        nc.gpsimd.memset(tb.ap(), SQPH)
        nc.const_aps.aps[(F32, SQPH)] = tb.ap()

    n_blocks = (rows + PB - 1) // PB
    wmax = halo_l + max(strips) + halo_r

    def product(eng, out, in0, in1):
        if isinstance(eng, (list, tuple)) and eng[0] == "split":
            frac = eng[1]
            m = out.shape[1]
            k = max(2, int(m * frac)) & ~1
            nc.gpsimd.tensor_tensor(out=out[:, 0:k], in0=in0[:, 0:k],
                                    in1=in1[:, 0:k], op=OP.mult)
            nc.vector.tensor_tensor(out=out[:, k:m], in0=in0[:, k:m],
                                    in1=in1[:, k:m], op=OP.mult)
            return
        e = nc.vector if eng == "dve" else nc.gpsimd
        e.tensor_tensor(out=out, in0=in0, in1=in1, op=OP.mult)

    def pick(eng, jidx, njobs):
        if isinstance(eng, str):
            return eng
        if isinstance(eng, (list, tuple)):
            mode, k = eng
            if mode == "split":
                return eng
            if mode == "head":
                return "dve" if jidx < k else "pool"
            if mode == "tail":
                return "dve" if jidx >= njobs - k else "pool"
            raise ValueError(eng)
        k = int(round(eng * njobs))
        return "pool" if jidx < k else "dve"

    with tile.TileContext(nc) as tc:
        with tc.tile_pool(name="fixed", bufs=1) as fixed:
            fb = fixed.tile([PB, n], BF16, tag="fb", name="t_fb")
            fb_pieces = [(ci * n // fb_chunks, (ci + 1) * n // fb_chunks)
                         for ci in range(fb_chunks)]

            perblk = []
            for blk in range(n_blocks):
                order = strips if blk % 2 == 0 else strips[::-1]
                pos = 0
                row = []
                for ssz in order:
                    row.append((blk * PB, pos, ssz))
                    pos += ssz
                perblk.append(row)
            jobs = [j for pair in zip(*perblk) for j in pair]

            def front(pool, r0, s, ssz, jidx, njobs):
                """alpha DMA, ACT squares, C, m3, g."""
                w = halo_l + ssz + halo_r
                dom_lo = max(0, min(s - halo_l, n - w))
                j = {
                    "r0": r0, "s": s, "oo": s - dom_lo, "w": w, "ssz": ssz,
                    "jidx": jidx, "njobs": njobs,
                    # padded tiles: reserved zero cols for shifted reads
                    "at": pool.tile([PB, wmax + 2], BF16, tag="at", name="t_at"),
                    "a2": pool.tile([PB, wmax + 2], BF16, tag="a2", name="t_a2"),
                    "ct": pool.tile([PB, wmax + 2], BF16, tag="ct", name="t_ct"),
                    "qt": pool.tile([PB, wmax + 2], BF16, tag="qt", name="t_qt"),
                    "gt": pool.tile([PB, wmax + 2], BF16, tag="gt", name="t_gt"),
                    "nr": pool.tile([PB, wmax], BF16, tag="nr", name="t_nr"),
                    "tt": pool.tile([PB, wmax + 2], BF16, tag="tt", name="t_tt"),
                }
                at, a2, ct, qt = j["at"], j["a2"], j["ct"], j["qt"]
                nc.sync.dma_start(out=at[:, 0:w],
                                  in_=alpha_d[r0:r0 + PB, dom_lo:dom_lo + w])
                nc.gpsimd.memset(a2[:, 0:1], 0.0)
                if nr_mode != "sub":
                    nc.gpsimd.memset(qt[:, 0:1], 0.0)
                nc.scalar.activation(a2[:, 1:w + 1], at[:, 0:w], ACT.Square,
                                     bias=0.0, scale=1.0)
                nc.scalar.activation(qt[:, 1:w + 1], at[:, 0:w], ACT.Square,
                                     bias=SQPH, scale=SQP)
                if c_mode == "act":
                    st = j["tt"]  # stage S in tt (dead until t)
                    nc.scalar.activation(st[:, 1:w + 1], at[:, 0:w], ACT.Square,
                                         bias=1.0, scale=1.0)
                    nc.scalar.activation(ct[:, 1:w + 1], st[:, 1:w + 1],
                                         ACT.Copy, bias=-1.0, scale=1.0)
                else:
                    nc.scalar.activation(ct[:, 1:w + 1], at[:, 0:w], ACT.Square,
                                         bias=1.0, scale=1.0)
                    nc.vector.tensor_scalar(out=ct[:, 1:w + 1], in0=ct[:, 1:w + 1],
                                            scalar1=-1.0, scalar2=None, op0=OP.add)
                if m3_mode == "act":
                    nc.scalar.activation(qt[:, 1:w + 1], qt[:, 1:w + 1], ACT.Copy,
                                         bias=R3 - 1.0, scale=1.0)
                nc.gpsimd.memset(j["at"][:, 0:1], 0.0)
                # zero the t-shift pad the y-scan reads (guards NaN garbage)
                nc.gpsimd.memset(j["tt"][:, w + 1:w + 2], 0.0)
                return j

            def st_prep(j):
                """m3 = Q+(r-1) [DVE TS] and g = A2[k-1]*C."""
                w, a2, ct, qt = j["w"], j["a2"], j["ct"], j["qt"]
                if m3_mode != "act":
                    nc.vector.tensor_scalar(out=qt[:, 1:w + 1], in0=qt[:, 1:w + 1],
                                            scalar1=R3 - 1.0, scalar2=None,
                                            op0=OP.add)
                eg = ("dve" if (j["jidx"] < 2 or j["jidx"] >= j["njobs"] - 4)
                      else pick(eng_g, j["jidx"], j["njobs"]))
                product(eg, j["gt"][:, 1:w + 1],
                        a2[:, 0:w], ct[:, 1:w + 1])

            def is_edge(j):
                return (j["jidx"] < lat_edge[0]
                        or j["jidx"] >= j["njobs"] - lat_edge[1])


            def st_nr(j):
                """nr = m3 + g*m3[-1] (2t) or forward scan."""
                w = j["w"]
                if nr_mode == "sub":
                    nc.vector.tensor_tensor(out=j["nr"][:, 0:w],
                                            in0=j["qt"][:, 1:w + 1],
                                            in1=j["gt"][:, 1:w + 1],
                                            op=OP.subtract)
                elif nr_mode == "2t" and not is_edge(j):
                    nc.vector.tensor_tensor(out=j["nr"][:, 0:w],
                                            in0=j["gt"][:, 1:w + 1],
                                            in1=j["qt"][:, 0:w], op=OP.mult)
                    nc.gpsimd.dma_start(out=j["nr"][:, 0:w],
                                        in_=j["qt"][:, 1:w + 1], accum_op=OP.add)
                else:
                    nc.vector.tensor_tensor_scan(
                        out=j["nr"][:, 0:w], data0=j["gt"][:, 1:w + 1],
                        data1=j["qt"][:, 1:w + 1],
                        initial=0.0, op0=OP.mult, op1=OP.add,
                    )

            def st_q(j):
                """q = A2*nr into gt (g dead); t = C*nr into tt."""
                w = j["w"]
                product("dve", j["gt"][:, 1:w + 1],
                        j["a2"][:, 1:w + 1], j["nr"][:, 0:w])
                product(pick(eng_t, j["jidx"], j["njobs"]), j["tt"][:, 1:w + 1],
                        j["ct"][:, 1:w + 1], j["nr"][:, 0:w])

            def st_w(j):
                """w = f + (q*f)[-1] (2t) into at (alpha dead), or scan."""
                w = j["w"]
                dom_lo = j["s"] - j["oo"]
                fbs = fb[:, dom_lo:dom_lo + w]
                nc.vector.tensor_tensor(out=j["at"][:, 1:w + 1],
                                        in0=j["gt"][:, 1:w + 1],
                                        in1=fbs, op=OP.mult)
                if not is_edge(j):
                    nc.gpsimd.dma_start(out=j["at"][:, 0:w], in_=fbs,
                                        accum_op=OP.add)
                else:
                    nc.vector.tensor_tensor(out=j["at"][:, 0:w],
                                            in0=j["at"][:, 0:w], in1=fbs,
                                            op=OP.add)

            def st_y(j):
                """backward scan: y_i = t_{i+1}*y_{i+1} - w_i, into qt."""
                w = j["w"]
                nc.vector.tensor_tensor_scan(
                    out=j["qt"][:, 0:w][:, ::-1],
                    data0=j["tt"][:, 2:w + 2][:, ::-1],
                    data1=j["at"][:, 0:w][:, ::-1],
                    initial=0.0, op0=OP.mult, op1=OP.subtract,
                )

            def st_u(j):
                """u = nr*y into ct (C dead), DMA out."""
                oo, s, r0, m = j["oo"], j["s"], j["r0"], j["ssz"]
                ut = j["ct"]
                eng = "dve" if is_edge(j) else pick(eng_u, j["jidx"], j["njobs"])
                product(eng, ut[:, 0:m],
                        j["nr"][:, oo:oo + m], j["qt"][:, oo:oo + m])
                nc.sync.dma_start(out=out_d[r0:r0 + PB, s:s + m], in_=ut[:, 0:m])

            stages = [st_prep, st_nr, st_q, st_w, st_y, st_u]
            with tc.tile_pool(name="jobs", bufs=bufs) as pool:
                live = []
                nj = len(jobs)
                pieces = list(fb_pieces)
                for k in range(nj + max(lags)):
                    if k < nj:
                        r0, s, ssz = jobs[k]
                        live.append(front(pool, r0, s, ssz, k, nj))
                    if pieces and k >= 1:
                        lo, hi = pieces.pop(0)
                        nc.sync.dma_start(out=fb[:, lo:hi], in_=fb_d[:, lo:hi])
                    for fn, lag in zip(stages, lags):
                        i = k - lag
                        if 0 <= i < nj:
                            fn(live[i])
    return nc


_cached = None


def _get_program():
    global _cached
    if _cached is None:
        nc = bacc.Bacc("TRN2", target_bir_lowering=False, debug=False)
        build_core_program(nc)
        nc.compile()
        _cached = nc
    return _cached


def _in_maps(alpha, f):
    alpha16 = np.ascontiguousarray(alpha.astype(bfloat16))
    fb = np.ascontiguousarray(
        np.broadcast_to(f.astype(bfloat16).reshape(1, N), (PB, N))
    )
    return [
        {"alpha": alpha16[c * RPC:(c + 1) * RPC], "fb": fb}
        for c in range(NCORES)
    ]


def kernel(alpha: np.ndarray, f: np.ndarray) -> np.ndarray:
    alpha = np.ascontiguousarray(alpha, dtype=np.float32)
    f = np.ascontiguousarray(f, dtype=np.float32)
    nc = _get_program()
    res = bass_utils.run_bass_kernel_spmd(nc, _in_maps(alpha, f),
                                          core_ids=list(range(NCORES)))
    out = np.concatenate([r["out"] for r in res.results], axis=0)
    return out.astype(np.float32)


if __name__ == "__main__":
    rng = np.random.default_rng(0)
    a = (0.3 * rng.random((B, N))).astype(np.float32)
    fv = rng.standard_normal(N).astype(np.float32)
    u = kernel(a, fv)
    print(u.shape, u.dtype, np.abs(u).max())


# revision 27
# speedup vs baseline: 1.3349x; 1.0135x over previous
"""Batched tridiagonal (Thomas) solve on 8 TRN2 NeuronCores.

System per row (alpha in [0, 0.3)):
    sub a_i = alpha_{i-1}^2, diag b_i = 1 + alpha_i^3,
    super c_i = CS_{i+1},  CS_j = alpha_j^2 + 2 alpha_j

Forward elimination is contraction-dominated (|g| <= 0.097, |q| <= 0.11
per step), so both forward recurrences collapse to closed forms
(numerically validated: end-to-end rel err ~7e-3 vs the 2e-2 budget):
    nr_i ~= m3_i - g_i                     (nr ~= -1/denom; 1/x ~= 2-x,
                                            m3 = b-2 via minimax-linear a^3)
    w_i  ~= f_i + (q*f)_{i-1}              (dp numerator, 2-term Neumann)
Only the backward substitution (decay 0.77/step) runs as a real
tensor_tensor_scan:  y_i = t_{i+1}*y_{i+1} - w_i,  u = nr*y.

Engine split per (128-row, strip) job, all bf16:
  ACT : A2 = a^2, S = (a+1)^2, C = Copy(S-1)
  DVE : m3 = L1*a+(L0-1) [TS], nr = m3-g [TT], products g/q/t1w/t
        [bf16 2x TT], the backward y-scan, edge-job forward scans
  Pool: u product, a column-split share of g, SWDGE issue of the w add
  DMA : alpha in, u out, shared-f broadcast, and the w-assembly "+f"
        via an accum-add DMA (dst += in) on the otherwise idle DMA fleet.
The first/last jobs ("edge") use true forward scans and DVE-only paths
to minimize pipeline fill/drain latency; interior jobs use the
throughput path above, software-pipelined via staged lags.

Sharding: pure data parallel over batch rows (256 rows/core = 2 blocks
of 128 partitions); columns split into strips with contraction halos so
every job is independent. f is shared: one bf16 [128, 8192] broadcast
load per core. Host does dtype casts and the final fp32 cast.
"""

import sys

sys.path.insert(0, "/opt/trn_rl_repo")

import numpy as np
from ml_dtypes import bfloat16

from concourse import bacc, mybir, tile
from concourse import bass_utils

F32 = mybir.dt.float32
BF16 = mybir.dt.bfloat16
OP = mybir.AluOpType
ACT = mybir.ActivationFunctionType

B, N = 2048, 8192
NCORES = 8
RPC = B // NCORES          # rows per core
PB = 128                   # partition block (rows per job)
HALO_L = 2                 # exact reach of the closed-form forward pass
HALO_R = 32                # backward-scan warmup (contraction <= 0.77/step)

# minimax fit alpha^3 ~= P3*(alpha+H3)^2 + R3 on [0, 0.3), max err 8.44e-4
P3 = 0.45
H3 = -0.05625
R3 = -0.00058007812
SQP = float(np.sqrt(P3))            # Q = Square(SQP*alpha + SQP*H3)
SQPH = float(np.float32(SQP * H3))

DEFAULT_STRIPS = (704, 1504, 1568, 1536, 1536, 1344)


def build_core_program(nc, rows=RPC, n=N, strips=DEFAULT_STRIPS,
                       halo_l=HALO_L, halo_r=HALO_R, bufs=8,
                       eng_g=("split", 0.25), eng_q="dve", eng_t="dve",
                       eng_u="pool",
                       nr_mode="sub", w_mode="2t",
                       c_mode="act", m3_mode="lin",
                       lags=(1, 1, 3, 3, 4, 5), fb_chunks=4, lat_edge=(1, 2)):
    assert sum(strips) == n
    alpha_d = nc.dram_tensor("alpha", [rows, n], BF16, kind="ExternalInput").ap()
    fb_d = nc.dram_tensor("fb", [PB, n], BF16, kind="ExternalInput").ap()
    out_d = nc.dram_tensor("out", [rows, n], BF16, kind="ExternalOutput").ap()

    if m3_mode not in ("lin", "actlin"):
        # bias const AP for the Q-square activation
        tb = nc.alloc_sbuf_tensor("const-q-bias", [128, 1], F32)
        nc.gpsimd.memset(tb.ap(), SQPH)
        nc.const_aps.aps[(F32, SQPH)] = tb.ap()

    n_blocks = (rows + PB - 1) // PB
    wmax = halo_l + max(strips) + halo_r

    def product(eng, out, in0, in1):
        if isinstance(eng, (list, tuple)) and eng[0] == "split":
            frac = eng[1]
            m = out.shape[1]
            k = max(2, int(m * frac)) & ~1
            nc.gpsimd.tensor_tensor(out=out[:, 0:k], in0=in0[:, 0:k],
                                    in1=in1[:, 0:k], op=OP.mult)
            nc.vector.tensor_tensor(out=out[:, k:m], in0=in0[:, k:m],
                                    in1=in1[:, k:m], op=OP.mult)
            return
        e = nc.vector if eng == "dve" else nc.gpsimd
        e.tensor_tensor(out=out, in0=in0, in1=in1, op=OP.mult)

    def pick(eng, jidx, njobs):
        if isinstance(eng, str):
            return eng
        if isinstance(eng, (list, tuple)):
            mode, k = eng
            if mode == "split":
                return eng
            if mode == "head":
                return "dve" if jidx < k else "pool"
            if mode == "tail":
                return "dve" if jidx >= njobs - k else "pool"
            raise ValueError(eng)
        k = int(round(eng * njobs))
        return "pool" if jidx < k else "dve"

    with tile.TileContext(nc) as tc:
        with tc.tile_pool(name="fixed", bufs=1) as fixed:
            fb = fixed.tile([PB, n], BF16, tag="fb", name="t_fb")
            fb_pieces = [(ci * n // fb_chunks, (ci + 1) * n // fb_chunks)
                         for ci in range(fb_chunks)]

            perblk = []
            for blk in range(n_blocks):
                order = strips if blk % 2 == 0 else strips[::-1]
                pos = 0
                row = []
                for ssz in order:
                    row.append((blk * PB, pos, ssz))
                    pos += ssz
                perblk.append(row)
            jobs = [j for pair in zip(*perblk) for j in pair]

            def front(pool, r0, s, ssz, jidx, njobs):
                """alpha DMA, ACT squares, C, m3, g."""
                w = halo_l + ssz + halo_r
                dom_lo = max(0, min(s - halo_l, n - w))
                j = {
                    "r0": r0, "s": s, "oo": s - dom_lo, "w": w, "ssz": ssz,
                    "jidx": jidx, "njobs": njobs,
                    # padded tiles: reserved zero cols for shifted reads
                    "at": pool.tile([PB, wmax + 2], BF16, tag="at", name="t_at"),
                    "a2": pool.tile([PB, wmax + 2], BF16, tag="a2", name="t_a2"),
                    "ct": pool.tile([PB, wmax + 2], BF16, tag="ct", name="t_ct"),
                    "qt": pool.tile([PB, wmax + 2], BF16, tag="qt", name="t_qt"),
                    "gt": pool.tile([PB, wmax + 2], BF16, tag="gt", name="t_gt"),
                    "nr": pool.tile([PB, wmax], BF16, tag="nr", name="t_nr"),
                    "tt": pool.tile([PB, wmax + 2], BF16, tag="tt", name="t_tt"),
                }
                at, a2, ct, qt = j["at"], j["a2"], j["ct"], j["qt"]
                nc.sync.dma_start(out=at[:, 0:w],
                                  in_=alpha_d[r0:r0 + PB, dom_lo:dom_lo + w])
                nc.gpsimd.memset(a2[:, 0:1], 0.0)
                if nr_mode != "sub":
                    nc.gpsimd.memset(qt[:, 0:1], 0.0)
                nc.scalar.activation(a2[:, 1:w + 1], at[:, 0:w], ACT.Square,
                                     bias=0.0, scale=1.0)
                nc.scalar.activation(qt[:, 1:w + 1], at[:, 0:w], ACT.Square,
                                     bias=SQPH, scale=SQP)
                if c_mode == "act":
                    st = j["tt"]  # stage S in tt (dead until t)
                    nc.scalar.activation(st[:, 1:w + 1], at[:, 0:w], ACT.Square,
                                         bias=1.0, scale=1.0)
                    nc.scalar.activation(ct[:, 1:w + 1], st[:, 1:w + 1],
                                         ACT.Copy, bias=-1.0, scale=1.0)
                else:
                    nc.scalar.activation(ct[:, 1:w + 1], at[:, 0:w], ACT.Square,
                                         bias=1.0, scale=1.0)
                    nc.vector.tensor_scalar(out=ct[:, 1:w + 1], in0=ct[:, 1:w + 1],
                                            scalar1=-1.0, scalar2=None, op0=OP.add)
                if m3_mode == "act":
                    nc.scalar.activation(qt[:, 1:w + 1], qt[:, 1:w + 1], ACT.Copy,
                                         bias=R3 - 1.0, scale=1.0)
                nc.gpsimd.memset(j["at"][:, 0:1], 0.0)
                # zero the t-shift pad the y-scan reads (guards NaN garbage)
                nc.gpsimd.memset(j["tt"][:, w + 1:w + 2], 0.0)
                return j

            def st_prep(j):
                """m3 = Q+(r-1) [DVE TS] and g = A2[k-1]*C."""
                w, a2, ct, qt = j["w"], j["a2"], j["ct"], j["qt"]
                if m3_mode != "act":
                    nc.vector.tensor_scalar(out=qt[:, 1:w + 1], in0=qt[:, 1:w + 1],
                                            scalar1=R3 - 1.0, scalar2=None,
                                            op0=OP.add)
                eg = ("dve" if (j["jidx"] < 2 or j["jidx"] >= j["njobs"] - 4)
                      else pick(eng_g, j["jidx"], j["njobs"]))
                product(eg, j["gt"][:, 1:w + 1],
                        a2[:, 0:w], ct[:, 1:w + 1])

            def is_edge(j):
                return (j["jidx"] < lat_edge[0]
                        or j["jidx"] >= j["njobs"] - lat_edge[1])


            def st_nr(j):
                """nr = m3 + g*m3[-1] (2t) or forward scan."""
                w = j["w"]
                if nr_mode == "sub":
                    nc.vector.tensor_tensor(out=j["nr"][:, 0:w],
                                            in0=j["qt"][:, 1:w + 1],
                                            in1=j["gt"][:, 1:w + 1],
                                            op=OP.subtract)
                elif nr_mode == "2t" and not is_edge(j):
                    nc.vector.tensor_tensor(out=j["nr"][:, 0:w],
                                            in0=j["gt"][:, 1:w + 1],
                                            in1=j["qt"][:, 0:w], op=OP.mult)
                    nc.gpsimd.dma_start(out=j["nr"][:, 0:w],
                                        in_=j["qt"][:, 1:w + 1], accum_op=OP.add)
                else:
                    nc.vector.tensor_tensor_scan(
                        out=j["nr"][:, 0:w], data0=j["gt"][:, 1:w + 1],
                        data1=j["qt"][:, 1:w + 1],
                        initial=0.0, op0=OP.mult, op1=OP.add,
                    )

            def st_q(j):
                """q = A2*nr into gt (g dead); t = C*nr into tt."""
                w = j["w"]
                product("dve", j["gt"][:, 1:w + 1],
                        j["a2"][:, 1:w + 1], j["nr"][:, 0:w])
                product(pick(eng_t, j["jidx"], j["njobs"]), j["tt"][:, 1:w + 1],
                        j["ct"][:, 1:w + 1], j["nr"][:, 0:w])

            def st_w(j):
                """w = f + (q*f)[-1] (2t) into at (alpha dead), or scan."""
                w = j["w"]
                dom_lo = j["s"] - j["oo"]
                fbs = fb[:, dom_lo:dom_lo + w]
                nc.vector.tensor_tensor(out=j["at"][:, 1:w + 1],
                                        in0=j["gt"][:, 1:w + 1],
                                        in1=fbs, op=OP.mult)
                if not is_edge(j):
                    nc.gpsimd.dma_start(out=j["at"][:, 0:w], in_=fbs,
                                        accum_op=OP.add)
                else:
                    nc.vector.tensor_tensor(out=j["at"][:, 0:w],
                                            in0=j["at"][:, 0:w], in1=fbs,
                                            op=OP.add)

            def st_y(j):
                """backward scan: y_i = t_{i+1}*y_{i+1} - w_i, into qt."""
                w = j["w"]
                nc.vector.tensor_tensor_scan(
                    out=j["qt"][:, 0:w][:, ::-1],
                    data0=j["tt"][:, 2:w + 2][:, ::-1],
                    data1=j["at"][:, 0:w][:, ::-1],
                    initial=0.0, op0=OP.mult, op1=OP.subtract,
                )

            def st_u(j):
                """u = nr*y into ct (C dead), DMA out."""
                oo, s, r0, m = j["oo"], j["s"], j["r0"], j["ssz"]
                ut = j["ct"]
                eng = "dve" if is_edge(j) else pick(eng_u, j["jidx"], j["njobs"])
                product(eng, ut[:, 0:m],
                        j["nr"][:, oo:oo + m], j["qt"][:, oo:oo + m])
                nc.sync.dma_start(out=out_d[r0:r0 + PB, s:s + m], in_=ut[:, 0:m])

            stages = [st_prep, st_nr, st_q, st_w, st_y, st_u]
            with tc.tile_pool(name="jobs", bufs=bufs) as pool:
                live = []
                nj = len(jobs)
                pieces = list(fb_pieces)
                for k in range(nj + max(lags)):
                    if k < nj:
                        r0, s, ssz = jobs[k]
                        live.append(front(pool, r0, s, ssz, k, nj))
                    if pieces and k >= 1:
                        lo, hi = pieces.pop(0)
                        nc.sync.dma_start(out=fb[:, lo:hi], in_=fb_d[:, lo:hi])
                    for fn, lag in zip(stages, lags):
                        i = k - lag
                        if 0 <= i < nj:
                            fn(live[i])
    return nc


_cached = None


def _get_program():
    global _cached
    if _cached is None:
        nc = bacc.Bacc("TRN2", target_bir_lowering=False, debug=False)
        build_core_program(nc)
        nc.compile()
        _cached = nc
    return _cached


def _in_maps(alpha, f):
    alpha16 = np.ascontiguousarray(alpha.astype(bfloat16))
    fb = np.ascontiguousarray(
        np.broadcast_to(f.astype(bfloat16).reshape(1, N), (PB, N))
    )
    return [
        {"alpha": alpha16[c * RPC:(c + 1) * RPC], "fb": fb}
        for c in range(NCORES)
    ]


def kernel(alpha: np.ndarray, f: np.ndarray) -> np.ndarray:
    alpha = np.ascontiguousarray(alpha, dtype=np.float32)
    f = np.ascontiguousarray(f, dtype=np.float32)
    nc = _get_program()
    res = bass_utils.run_bass_kernel_spmd(nc, _in_maps(alpha, f),
                                          core_ids=list(range(NCORES)))
    out = np.concatenate([r["out"] for r in res.results], axis=0)
    return out.astype(np.float32)


if __name__ == "__main__":
    rng = np.random.default_rng(0)
    a = (0.3 * rng.random((B, N))).astype(np.float32)
    fv = rng.standard_normal(N).astype(np.float32)
    u = kernel(a, fv)
    print(u.shape, u.dtype, np.abs(u).max())


# revision 28
# speedup vs baseline: 1.3406x; 1.0043x over previous
"""Batched tridiagonal (Thomas) solve on 8 TRN2 NeuronCores.

System per row (alpha in [0, 0.3)):
    sub a_i = alpha_{i-1}^2, diag b_i = 1 + alpha_i^3,
    super c_i = CS_{i+1},  CS_j = alpha_j^2 + 2 alpha_j

Forward elimination is contraction-dominated (|g| <= 0.097, |q| <= 0.11
per step), so both forward recurrences collapse to closed forms
(numerically validated: end-to-end rel err ~7e-3 vs the 2e-2 budget):
    nr_i ~= m3_i - g_i                     (nr ~= -1/denom; 1/x ~= 2-x,
                                            m3 = b-2 via minimax-linear a^3)
    w_i  ~= f_i + (q*f)_{i-1}              (dp numerator, 2-term Neumann)
Only the backward substitution (decay 0.77/step) runs as a real
tensor_tensor_scan:  y_i = t_{i+1}*y_{i+1} - w_i,  u = nr*y.

Engine split per (128-row, strip) job, all bf16:
  ACT : A2 = a^2, S = (a+1)^2, C = Copy(S-1)
  DVE : m3 = L1*a+(L0-1) [TS], nr = m3-g [TT], products g/q/t1w/t
        [bf16 2x TT], the backward y-scan, edge-job forward scans
  Pool: u product, a column-split share of g, SWDGE issue of the w add
  DMA : alpha in, u out, shared-f broadcast, and the w-assembly "+f"
        via an accum-add DMA (dst += in) on the otherwise idle DMA fleet.
The first/last jobs ("edge") use true forward scans and DVE-only paths
to minimize pipeline fill/drain latency; interior jobs use the
throughput path above, software-pipelined via staged lags.

Sharding: pure data parallel over batch rows (256 rows/core = 2 blocks
of 128 partitions); columns split into strips with contraction halos so
every job is independent. f is shared: one bf16 [128, 8192] broadcast
load per core. Host does dtype casts and the final fp32 cast.
"""

import sys

sys.path.insert(0, "/opt/trn_rl_repo")

import numpy as np
from ml_dtypes import bfloat16

from concourse import bacc, mybir, tile
from concourse import bass_utils

F32 = mybir.dt.float32
BF16 = mybir.dt.bfloat16
OP = mybir.AluOpType
ACT = mybir.ActivationFunctionType

B, N = 2048, 8192
NCORES = 8
RPC = B // NCORES          # rows per core
PB = 128                   # partition block (rows per job)
HALO_L = 2                 # exact reach of the closed-form forward pass
HALO_R = 24                # backward-scan warmup (contraction <= 0.77/step)

# minimax fit alpha^3 ~= P3*(alpha+H3)^2 + R3 on [0, 0.3), max err 8.44e-4
P3 = 0.45
H3 = -0.05625
R3 = -0.00058007812
SQP = float(np.sqrt(P3))            # Q = Square(SQP*alpha + SQP*H3)
SQPH = float(np.float32(SQP * H3))

DEFAULT_STRIPS = (704, 1504, 1568, 1536, 1536, 1344)


def build_core_program(nc, rows=RPC, n=N, strips=DEFAULT_STRIPS,
                       halo_l=HALO_L, halo_r=HALO_R, bufs=8,
                       eng_g=("split", 0.25), eng_q="dve", eng_t="dve",
                       eng_u="pool",
                       nr_mode="sub", w_mode="2t",
                       c_mode="act", m3_mode="lin",
                       lags=(1, 1, 3, 3, 4, 5), fb_chunks=4, lat_edge=(1, 2)):
    assert sum(strips) == n
    alpha_d = nc.dram_tensor("alpha", [rows, n], BF16, kind="ExternalInput").ap()
    fb_d = nc.dram_tensor("fb", [PB, n], BF16, kind="ExternalInput").ap()
    out_d = nc.dram_tensor("out", [rows, n], BF16, kind="ExternalOutput").ap()

    if m3_mode not in ("lin", "actlin"):
        # bias const AP for the Q-square activation
        tb = nc.alloc_sbuf_tensor("const-q-bias", [128, 1], F32)
        nc.gpsimd.memset(tb.ap(), SQPH)
        nc.const_aps.aps[(F32, SQPH)] = tb.ap()

    n_blocks = (rows + PB - 1) // PB
    wmax = halo_l + max(strips) + halo_r

    def product(eng, out, in0, in1):
        if isinstance(eng, (list, tuple)) and eng[0] == "split":
            frac = eng[1]
            m = out.shape[1]
            k = max(2, int(m * frac)) & ~1
            nc.gpsimd.tensor_tensor(out=out[:, 0:k], in0=in0[:, 0:k],
                                    in1=in1[:, 0:k], op=OP.mult)
            nc.vector.tensor_tensor(out=out[:, k:m], in0=in0[:, k:m],
                                    in1=in1[:, k:m], op=OP.mult)
            return
        e = nc.vector if eng == "dve" else nc.gpsimd
        e.tensor_tensor(out=out, in0=in0, in1=in1, op=OP.mult)

    def pick(eng, jidx, njobs):
        if isinstance(eng, str):
            return eng
        if isinstance(eng, (list, tuple)):
            mode, k = eng
            if mode == "split":
                return eng
            if mode == "head":
                return "dve" if jidx < k else "pool"
            if mode == "tail":
                return "dve" if jidx >= njobs - k else "pool"
            raise ValueError(eng)
        k = int(round(eng * njobs))
        return "pool" if jidx < k else "dve"

    with tile.TileContext(nc) as tc:
        with tc.tile_pool(name="fixed", bufs=1) as fixed:
            fb = fixed.tile([PB, n], BF16, tag="fb", name="t_fb")
            fb_pieces = [(ci * n // fb_chunks, (ci + 1) * n // fb_chunks)
                         for ci in range(fb_chunks)]

            perblk = []
            for blk in range(n_blocks):
                order = strips if blk % 2 == 0 else strips[::-1]
                pos = 0
                row = []
                for ssz in order:
                    row.append((blk * PB, pos, ssz))
                    pos += ssz
                perblk.append(row)
            jobs = [j for pair in zip(*perblk) for j in pair]

            def front(pool, r0, s, ssz, jidx, njobs):
                """alpha DMA, ACT squares, C, m3, g."""
                w = halo_l + ssz + halo_r
                dom_lo = max(0, min(s - halo_l, n - w))
                j = {
                    "r0": r0, "s": s, "oo": s - dom_lo, "w": w, "ssz": ssz,
                    "jidx": jidx, "njobs": njobs,
                    # padded tiles: reserved zero cols for shifted reads
                    "at": pool.tile([PB, wmax + 2], BF16, tag="at", name="t_at"),
                    "a2": pool.tile([PB, wmax + 2], BF16, tag="a2", name="t_a2"),
                    "ct": pool.tile([PB, wmax + 2], BF16, tag="ct", name="t_ct"),
                    "qt": pool.tile([PB, wmax + 2], BF16, tag="qt", name="t_qt"),
                    "gt": pool.tile([PB, wmax + 2], BF16, tag="gt", name="t_gt"),
                    "nr": pool.tile([PB, wmax], BF16, tag="nr", name="t_nr"),
                    "tt": pool.tile([PB, wmax + 2], BF16, tag="tt", name="t_tt"),
                }
                at, a2, ct, qt = j["at"], j["a2"], j["ct"], j["qt"]
                nc.sync.dma_start(out=at[:, 0:w],
                                  in_=alpha_d[r0:r0 + PB, dom_lo:dom_lo + w])
                nc.gpsimd.memset(a2[:, 0:1], 0.0)
                if nr_mode != "sub":
                    nc.gpsimd.memset(qt[:, 0:1], 0.0)
                nc.scalar.activation(a2[:, 1:w + 1], at[:, 0:w], ACT.Square,
                                     bias=0.0, scale=1.0)
                nc.scalar.activation(qt[:, 1:w + 1], at[:, 0:w], ACT.Square,
                                     bias=SQPH, scale=SQP)
                if c_mode == "act":
                    st = j["tt"]  # stage S in tt (dead until t)
                    nc.scalar.activation(st[:, 1:w + 1], at[:, 0:w], ACT.Square,
                                         bias=1.0, scale=1.0)
                    nc.scalar.activation(ct[:, 1:w + 1], st[:, 1:w + 1],
                                         ACT.Copy, bias=-1.0, scale=1.0)
                else:
                    nc.scalar.activation(ct[:, 1:w + 1], at[:, 0:w], ACT.Square,
                                         bias=1.0, scale=1.0)
                    nc.vector.tensor_scalar(out=ct[:, 1:w + 1], in0=ct[:, 1:w + 1],
                                            scalar1=-1.0, scalar2=None, op0=OP.add)
                if m3_mode == "act":
                    nc.scalar.activation(qt[:, 1:w + 1], qt[:, 1:w + 1], ACT.Copy,
                                         bias=R3 - 1.0, scale=1.0)
                nc.gpsimd.memset(j["at"][:, 0:1], 0.0)
                # zero the t-shift pad the y-scan reads (guards NaN garbage)
                nc.gpsimd.memset(j["tt"][:, w + 1:w + 2], 0.0)
                return j

            def st_prep(j):
                """m3 = Q+(r-1) [DVE TS] and g = A2[k-1]*C."""
                w, a2, ct, qt = j["w"], j["a2"], j["ct"], j["qt"]
                if m3_mode != "act":
                    nc.vector.tensor_scalar(out=qt[:, 1:w + 1], in0=qt[:, 1:w + 1],
                                            scalar1=R3 - 1.0, scalar2=None,
                                            op0=OP.add)
                eg = ("dve" if (j["jidx"] < 2 or j["jidx"] >= j["njobs"] - 4)
                      else pick(eng_g, j["jidx"], j["njobs"]))
                product(eg, j["gt"][:, 1:w + 1],
                        a2[:, 0:w], ct[:, 1:w + 1])

            def is_edge(j):
                return (j["jidx"] < lat_edge[0]
                        or j["jidx"] >= j["njobs"] - lat_edge[1])


            def st_nr(j):
                """nr = m3 + g*m3[-1] (2t) or forward scan."""
                w = j["w"]
                if nr_mode == "sub":
                    nc.vector.tensor_tensor(out=j["nr"][:, 0:w],
                                            in0=j["qt"][:, 1:w + 1],
                                            in1=j["gt"][:, 1:w + 1],
                                            op=OP.subtract)
                elif nr_mode == "2t" and not is_edge(j):
                    nc.vector.tensor_tensor(out=j["nr"][:, 0:w],
                                            in0=j["gt"][:, 1:w + 1],
                                            in1=j["qt"][:, 0:w], op=OP.mult)
                    nc.gpsimd.dma_start(out=j["nr"][:, 0:w],
                                        in_=j["qt"][:, 1:w + 1], accum_op=OP.add)
                else:
                    nc.vector.tensor_tensor_scan(
                        out=j["nr"][:, 0:w], data0=j["gt"][:, 1:w + 1],
                        data1=j["qt"][:, 1:w + 1],
                        initial=0.0, op0=OP.mult, op1=OP.add,
                    )

            def st_q(j):
                """q = A2*nr into gt (g dead); t = C*nr into tt."""
                w = j["w"]
                product("dve", j["gt"][:, 1:w + 1],
                        j["a2"][:, 1:w + 1], j["nr"][:, 0:w])
                product(pick(eng_t, j["jidx"], j["njobs"]), j["tt"][:, 1:w + 1],
                        j["ct"][:, 1:w + 1], j["nr"][:, 0:w])

            def st_w(j):
                """w = f + (q*f)[-1] (2t) into at (alpha dead), or scan."""
                w = j["w"]
                dom_lo = j["s"] - j["oo"]
                fbs = fb[:, dom_lo:dom_lo + w]
                nc.vector.tensor_tensor(out=j["at"][:, 1:w + 1],
                                        in0=j["gt"][:, 1:w + 1],
                                        in1=fbs, op=OP.mult)
                if not is_edge(j):
                    nc.gpsimd.dma_start(out=j["at"][:, 0:w], in_=fbs,
                                        accum_op=OP.add)
                else:
                    nc.vector.tensor_tensor(out=j["at"][:, 0:w],
                                            in0=j["at"][:, 0:w], in1=fbs,
                                            op=OP.add)

            def st_y(j):
                """backward scan: y_i = t_{i+1}*y_{i+1} - w_i, into qt."""
                w = j["w"]
                nc.vector.tensor_tensor_scan(
                    out=j["qt"][:, 0:w][:, ::-1],
                    data0=j["tt"][:, 2:w + 2][:, ::-1],
                    data1=j["at"][:, 0:w][:, ::-1],
                    initial=0.0, op0=OP.mult, op1=OP.subtract,
                )

            def st_u(j):
                """u = nr*y into ct (C dead), DMA out."""
                oo, s, r0, m = j["oo"], j["s"], j["r0"], j["ssz"]
                ut = j["ct"]
                eng = "dve" if is_edge(j) else pick(eng_u, j["jidx"], j["njobs"])
                product(eng, ut[:, 0:m],
                        j["nr"][:, oo:oo + m], j["qt"][:, oo:oo + m])
                nc.sync.dma_start(out=out_d[r0:r0 + PB, s:s + m], in_=ut[:, 0:m])

            stages = [st_prep, st_nr, st_q, st_w, st_y, st_u]
            with tc.tile_pool(name="jobs", bufs=bufs) as pool:
                live = []
                nj = len(jobs)
                pieces = list(fb_pieces)
                for k in range(nj + max(lags)):
                    if k < nj:
                        r0, s, ssz = jobs[k]
                        live.append(front(pool, r0, s, ssz, k, nj))
                    if pieces and k >= 1:
                        lo, hi = pieces.pop(0)
                        nc.sync.dma_start(out=fb[:, lo:hi], in_=fb_d[:, lo:hi])
                    for fn, lag in zip(stages, lags):
                        i = k - lag
                        if 0 <= i < nj:
                            fn(live[i])
    return nc


_cached = None


def _get_program():
    global _cached
    if _cached is None:
        nc = bacc.Bacc("TRN2", target_bir_lowering=False, debug=False)
        build_core_program(nc)
        nc.compile()
        _cached = nc
    return _cached


def _in_maps(alpha, f):
    alpha16 = np.ascontiguousarray(alpha.astype(bfloat16))
    fb = np.ascontiguousarray(
        np.broadcast_to(f.astype(bfloat16).reshape(1, N), (PB, N))
    )
    return [
        {"alpha": alpha16[c * RPC:(c + 1) * RPC], "fb": fb}
        for c in range(NCORES)
    ]


def kernel(alpha: np.ndarray, f: np.ndarray) -> np.ndarray:
    alpha = np.ascontiguousarray(alpha, dtype=np.float32)
    f = np.ascontiguousarray(f, dtype=np.float32)
    nc = _get_program()
    res = bass_utils.run_bass_kernel_spmd(nc, _in_maps(alpha, f),
                                          core_ids=list(range(NCORES)))
    out = np.concatenate([r["out"] for r in res.results], axis=0)
    return out.astype(np.float32)


if __name__ == "__main__":
    rng = np.random.default_rng(0)
    a = (0.3 * rng.random((B, N))).astype(np.float32)
    fv = rng.standard_normal(N).astype(np.float32)
    u = kernel(a, fv)
    print(u.shape, u.dtype, np.abs(u).max())


# revision 29
# speedup vs baseline: 1.3463x; 1.0042x over previous
"""Batched tridiagonal (Thomas) solve on 8 TRN2 NeuronCores.

System per row (alpha in [0, 0.3)):
    sub a_i = alpha_{i-1}^2, diag b_i = 1 + alpha_i^3,
    super c_i = CS_{i+1},  CS_j = alpha_j^2 + 2 alpha_j

Forward elimination is contraction-dominated (|g| <= 0.097, |q| <= 0.11
per step), so both forward recurrences collapse to closed forms
(numerically validated: end-to-end rel err ~7e-3 vs the 2e-2 budget):
    nr_i ~= m3_i - g_i                     (nr ~= -1/denom; 1/x ~= 2-x,
                                            m3 = b-2 via minimax-linear a^3)
    w_i  ~= f_i + (q*f)_{i-1}              (dp numerator, 2-term Neumann)
Only the backward substitution (decay 0.77/step) runs as a real
tensor_tensor_scan:  y_i = t_{i+1}*y_{i+1} - w_i,  u = nr*y.

Engine split per (128-row, strip) job, all bf16:
  ACT : A2 = a^2, S = (a+1)^2, C = Copy(S-1)
  DVE : m3 = L1*a+(L0-1) [TS], nr = m3-g [TT], products g/q/t1w/t
        [bf16 2x TT], the backward y-scan, edge-job forward scans
  Pool: u product, a column-split share of g, SWDGE issue of the w add
  DMA : alpha in, u out, shared-f broadcast, and the w-assembly "+f"
        via an accum-add DMA (dst += in) on the otherwise idle DMA fleet.
The first/last jobs ("edge") use true forward scans and DVE-only paths
to minimize pipeline fill/drain latency; interior jobs use the
throughput path above, software-pipelined via staged lags.

Sharding: pure data parallel over batch rows (256 rows/core = 2 blocks
of 128 partitions); columns split into strips with contraction halos so
every job is independent. f is shared: one bf16 [128, 8192] broadcast
load per core. Host does dtype casts and the final fp32 cast.
"""

import sys

sys.path.insert(0, "/opt/trn_rl_repo")

import numpy as np
from ml_dtypes import bfloat16

from concourse import bacc, mybir, tile
from concourse import bass_utils

F32 = mybir.dt.float32
BF16 = mybir.dt.bfloat16
OP = mybir.AluOpType
ACT = mybir.ActivationFunctionType

B, N = 2048, 8192
NCORES = 8
RPC = B // NCORES          # rows per core
PB = 128                   # partition block (rows per job)
HALO_L = 2                 # exact reach of the closed-form forward pass
HALO_R = 16                # backward-scan warmup (contraction <= 0.77/step)

# minimax fit alpha^3 ~= P3*(alpha+H3)^2 + R3 on [0, 0.3), max err 8.44e-4
P3 = 0.45
H3 = -0.05625
R3 = -0.00058007812
SQP = float(np.sqrt(P3))            # Q = Square(SQP*alpha + SQP*H3)
SQPH = float(np.float32(SQP * H3))

DEFAULT_STRIPS = (704, 1504, 1568, 1536, 1536, 1344)


def build_core_program(nc, rows=RPC, n=N, strips=DEFAULT_STRIPS,
                       halo_l=HALO_L, halo_r=HALO_R, bufs=8,
                       eng_g=("split", 0.25), eng_q="dve", eng_t="dve",
                       eng_u="pool",
                       nr_mode="sub", w_mode="2t",
                       c_mode="act", m3_mode="lin",
                       lags=(1, 1, 3, 3, 4, 5), fb_chunks=4, lat_edge=(1, 2)):
    assert sum(strips) == n
    alpha_d = nc.dram_tensor("alpha", [rows, n], BF16, kind="ExternalInput").ap()
    fb_d = nc.dram_tensor("fb", [PB, n], BF16, kind="ExternalInput").ap()
    out_d = nc.dram_tensor("out", [rows, n], BF16, kind="ExternalOutput").ap()

    if m3_mode not in ("lin", "actlin"):
        # bias const AP for the Q-square activation
        tb = nc.alloc_sbuf_tensor("const-q-bias", [128, 1], F32)
        nc.gpsimd.memset(tb.ap(), SQPH)
        nc.const_aps.aps[(F32, SQPH)] = tb.ap()

    n_blocks = (rows + PB - 1) // PB
    wmax = halo_l + max(strips) + halo_r

    def product(eng, out, in0, in1):
        if isinstance(eng, (list, tuple)) and eng[0] == "split":
            frac = eng[1]
            m = out.shape[1]
            k = max(2, int(m * frac)) & ~1
            nc.gpsimd.tensor_tensor(out=out[:, 0:k], in0=in0[:, 0:k],
                                    in1=in1[:, 0:k], op=OP.mult)
            nc.vector.tensor_tensor(out=out[:, k:m], in0=in0[:, k:m],
                                    in1=in1[:, k:m], op=OP.mult)
            return
        e = nc.vector if eng == "dve" else nc.gpsimd
        e.tensor_tensor(out=out, in0=in0, in1=in1, op=OP.mult)

    def pick(eng, jidx, njobs):
        if isinstance(eng, str):
            return eng
        if isinstance(eng, (list, tuple)):
            mode, k = eng
            if mode == "split":
                return eng
            if mode == "head":
                return "dve" if jidx < k else "pool"
            if mode == "tail":
                return "dve" if jidx >= njobs - k else "pool"
            raise ValueError(eng)
        k = int(round(eng * njobs))
        return "pool" if jidx < k else "dve"

    with tile.TileContext(nc) as tc:
        with tc.tile_pool(name="fixed", bufs=1) as fixed:
            fb = fixed.tile([PB, n], BF16, tag="fb", name="t_fb")
            fb_pieces = [(ci * n // fb_chunks, (ci + 1) * n // fb_chunks)
                         for ci in range(fb_chunks)]

            perblk = []
            for blk in range(n_blocks):
                order = strips if blk % 2 == 0 else strips[::-1]
                pos = 0
                row = []
                for ssz in order:
                    row.append((blk * PB, pos, ssz))
                    pos += ssz
                perblk.append(row)
            jobs = [j for pair in zip(*perblk) for j in pair]

            def front(pool, r0, s, ssz, jidx, njobs):
                """alpha DMA, ACT squares, C, m3, g."""
                w = halo_l + ssz + halo_r
                dom_lo = max(0, min(s - halo_l, n - w))
                j = {
                    "r0": r0, "s": s, "oo": s - dom_lo, "w": w, "ssz": ssz,
                    "jidx": jidx, "njobs": njobs,
                    # padded tiles: reserved zero cols for shifted reads
                    "at": pool.tile([PB, wmax + 2], BF16, tag="at", name="t_at"),
                    "a2": pool.tile([PB, wmax + 2], BF16, tag="a2", name="t_a2"),
                    "ct": pool.tile([PB, wmax + 2], BF16, tag="ct", name="t_ct"),
                    "qt": pool.tile([PB, wmax + 2], BF16, tag="qt", name="t_qt"),
                    "gt": pool.tile([PB, wmax + 2], BF16, tag="gt", name="t_gt"),
                    "nr": pool.tile([PB, wmax], BF16, tag="nr", name="t_nr"),
                    "tt": pool.tile([PB, wmax + 2], BF16, tag="tt", name="t_tt"),
                }
                at, a2, ct, qt = j["at"], j["a2"], j["ct"], j["qt"]
                nc.sync.dma_start(out=at[:, 0:w],
                                  in_=alpha_d[r0:r0 + PB, dom_lo:dom_lo + w])
                nc.gpsimd.memset(a2[:, 0:1], 0.0)
                if nr_mode != "sub":
                    nc.gpsimd.memset(qt[:, 0:1], 0.0)
                nc.scalar.activation(a2[:, 1:w + 1], at[:, 0:w], ACT.Square,
                                     bias=0.0, scale=1.0)
                nc.scalar.activation(qt[:, 1:w + 1], at[:, 0:w], ACT.Square,
                                     bias=SQPH, scale=SQP)
                if c_mode == "act":
                    st = j["tt"]  # stage S in tt (dead until t)
                    nc.scalar.activation(st[:, 1:w + 1], at[:, 0:w], ACT.Square,
                                         bias=1.0, scale=1.0)
                    nc.scalar.activation(ct[:, 1:w + 1], st[:, 1:w + 1],
                                         ACT.Copy, bias=-1.0, scale=1.0)
                else:
                    nc.scalar.activation(ct[:, 1:w + 1], at[:, 0:w], ACT.Square,
                                         bias=1.0, scale=1.0)
                    nc.vector.tensor_scalar(out=ct[:, 1:w + 1], in0=ct[:, 1:w + 1],
                                            scalar1=-1.0, scalar2=None, op0=OP.add)
                if m3_mode == "act":
                    nc.scalar.activation(qt[:, 1:w + 1], qt[:, 1:w + 1], ACT.Copy,
                                         bias=R3 - 1.0, scale=1.0)
                nc.gpsimd.memset(j["at"][:, 0:1], 0.0)
                # zero the t-shift pad the y-scan reads (guards NaN garbage)
                nc.gpsimd.memset(j["tt"][:, w + 1:w + 2], 0.0)
                return j

            def st_prep(j):
                """m3 = Q+(r-1) [DVE TS] and g = A2[k-1]*C."""
                w, a2, ct, qt = j["w"], j["a2"], j["ct"], j["qt"]
                if m3_mode != "act":
                    nc.vector.tensor_scalar(out=qt[:, 1:w + 1], in0=qt[:, 1:w + 1],
                                            scalar1=R3 - 1.0, scalar2=None,
                                            op0=OP.add)
                eg = ("dve" if (j["jidx"] < 2 or j["jidx"] >= j["njobs"] - 4)
                      else pick(eng_g, j["jidx"], j["njobs"]))
                product(eg, j["gt"][:, 1:w + 1],
                        a2[:, 0:w], ct[:, 1:w + 1])

            def is_edge(j):
                return (j["jidx"] < lat_edge[0]
                        or j["jidx"] >= j["njobs"] - lat_edge[1])


            def st_nr(j):
                """nr = m3 + g*m3[-1] (2t) or forward scan."""
                w = j["w"]
                if nr_mode == "sub":
                    nc.vector.tensor_tensor(out=j["nr"][:, 0:w],
                                            in0=j["qt"][:, 1:w + 1],
                                            in1=j["gt"][:, 1:w + 1],
                                            op=OP.subtract)
                elif nr_mode == "2t" and not is_edge(j):
                    nc.vector.tensor_tensor(out=j["nr"][:, 0:w],
                                            in0=j["gt"][:, 1:w + 1],
                                            in1=j["qt"][:, 0:w], op=OP.mult)
                    nc.gpsimd.dma_start(out=j["nr"][:, 0:w],
                                        in_=j["qt"][:, 1:w + 1], accum_op=OP.add)
                else:
                    nc.vector.tensor_tensor_scan(
                        out=j["nr"][:, 0:w], data0=j["gt"][:, 1:w + 1],
                        data1=j["qt"][:, 1:w + 1],
                        initial=0.0, op0=OP.mult, op1=OP.add,
                    )

            def st_q(j):
                """q = A2*nr into gt (g dead); t = C*nr into tt."""
                w = j["w"]
                product("dve", j["gt"][:, 1:w + 1],
                        j["a2"][:, 1:w + 1], j["nr"][:, 0:w])
                product(pick(eng_t, j["jidx"], j["njobs"]), j["tt"][:, 1:w + 1],
                        j["ct"][:, 1:w + 1], j["nr"][:, 0:w])

            def st_w(j):
                """w = f + (q*f)[-1] (2t) into at (alpha dead), or scan."""
                w = j["w"]
                dom_lo = j["s"] - j["oo"]
                fbs = fb[:, dom_lo:dom_lo + w]
                nc.vector.tensor_tensor(out=j["at"][:, 1:w + 1],
                                        in0=j["gt"][:, 1:w + 1],
                                        in1=fbs, op=OP.mult)
                if not is_edge(j):
                    nc.gpsimd.dma_start(out=j["at"][:, 0:w], in_=fbs,
                                        accum_op=OP.add)
                else:
                    nc.vector.tensor_tensor(out=j["at"][:, 0:w],
                                            in0=j["at"][:, 0:w], in1=fbs,
                                            op=OP.add)

            def st_y(j):
                """backward scan: y_i = t_{i+1}*y_{i+1} - w_i, into qt."""
                w = j["w"]
                nc.vector.tensor_tensor_scan(
                    out=j["qt"][:, 0:w][:, ::-1],
                    data0=j["tt"][:, 2:w + 2][:, ::-1],
                    data1=j["at"][:, 0:w][:, ::-1],
                    initial=0.0, op0=OP.mult, op1=OP.subtract,
                )

            def st_u(j):
                """u = nr*y into ct (C dead), DMA out."""
                oo, s, r0, m = j["oo"], j["s"], j["r0"], j["ssz"]
                ut = j["ct"]
                eng = "dve" if is_edge(j) else pick(eng_u, j["jidx"], j["njobs"])
                product(eng, ut[:, 0:m],
                        j["nr"][:, oo:oo + m], j["qt"][:, oo:oo + m])
                nc.sync.dma_start(out=out_d[r0:r0 + PB, s:s + m], in_=ut[:, 0:m])

            stages = [st_prep, st_nr, st_q, st_w, st_y, st_u]
            with tc.tile_pool(name="jobs", bufs=bufs) as pool:
                live = []
                nj = len(jobs)
                pieces = list(fb_pieces)
                for k in range(nj + max(lags)):
                    if k < nj:
                        r0, s, ssz = jobs[k]
                        live.append(front(pool, r0, s, ssz, k, nj))
                    if pieces and k >= 1:
                        lo, hi = pieces.pop(0)
                        nc.sync.dma_start(out=fb[:, lo:hi], in_=fb_d[:, lo:hi])
                    for fn, lag in zip(stages, lags):
                        i = k - lag
                        if 0 <= i < nj:
                            fn(live[i])
    return nc


_cached = None


def _get_program():
    global _cached
    if _cached is None:
        nc = bacc.Bacc("TRN2", target_bir_lowering=False, debug=False)
        build_core_program(nc)
        nc.compile()
        _cached = nc
    return _cached


def _in_maps(alpha, f):
    alpha16 = np.ascontiguousarray(alpha.astype(bfloat16))
    fb = np.ascontiguousarray(
        np.broadcast_to(f.astype(bfloat16).reshape(1, N), (PB, N))
    )
    return [
        {"alpha": alpha16[c * RPC:(c + 1) * RPC], "fb": fb}
        for c in range(NCORES)
    ]


def kernel(alpha: np.ndarray, f: np.ndarray) -> np.ndarray:
    alpha = np.ascontiguousarray(alpha, dtype=np.float32)
    f = np.ascontiguousarray(f, dtype=np.float32)
    nc = _get_program()
    res = bass_utils.run_bass_kernel_spmd(nc, _in_maps(alpha, f),
                                          core_ids=list(range(NCORES)))
    out = np.concatenate([r["out"] for r in res.results], axis=0)
    return out.astype(np.float32)


if __name__ == "__main__":
    rng = np.random.default_rng(0)
    a = (0.3 * rng.random((B, N))).astype(np.float32)
    fv = rng.standard_normal(N).astype(np.float32)
    u = kernel(a, fv)
    print(u.shape, u.dtype, np.abs(u).max())


# revision 30
# speedup vs baseline: 1.3490x; 1.0020x over previous
"""Batched tridiagonal (Thomas) solve on 8 TRN2 NeuronCores.

System per row (alpha in [0, 0.3)):
    sub a_i = alpha_{i-1}^2, diag b_i = 1 + alpha_i^3,
    super c_i = CS_{i+1},  CS_j = alpha_j^2 + 2 alpha_j

Forward elimination is contraction-dominated (|g| <= 0.097, |q| <= 0.11
per step), so both forward recurrences collapse to closed forms
(numerically validated: end-to-end rel err ~7e-3 vs the 2e-2 budget):
    nr_i ~= m3_i - g_i                     (nr ~= -1/denom; 1/x ~= 2-x,
                                            m3 = b-2 via minimax-linear a^3)
    w_i  ~= f_i + (q*f)_{i-1}              (dp numerator, 2-term Neumann)
Only the backward substitution (decay 0.77/step) runs as a real
tensor_tensor_scan:  y_i = t_{i+1}*y_{i+1} - w_i,  u = nr*y.

Engine split per (128-row, strip) job, all bf16:
  ACT : A2 = a^2, S = (a+1)^2, C = Copy(S-1)
  DVE : m3 = L1*a+(L0-1) [TS], nr = m3-g [TT], products g/q/t1w/t
        [bf16 2x TT], the backward y-scan, edge-job forward scans
  Pool: u product, a column-split share of g, SWDGE issue of the w add
  DMA : alpha in, u out, shared-f broadcast, and the w-assembly "+f"
        via an accum-add DMA (dst += in) on the otherwise idle DMA fleet.
The first/last jobs ("edge") use true forward scans and DVE-only paths
to minimize pipeline fill/drain latency; interior jobs use the
throughput path above, software-pipelined via staged lags.

Sharding: pure data parallel over batch rows (256 rows/core = 2 blocks
of 128 partitions); columns split into strips with contraction halos so
every job is independent. f is shared: one bf16 [128, 8192] broadcast
load per core. Host does dtype casts and the final fp32 cast.
"""

import sys

sys.path.insert(0, "/opt/trn_rl_repo")

import numpy as np
from ml_dtypes import bfloat16

from concourse import bacc, mybir, tile
from concourse import bass_utils

F32 = mybir.dt.float32
BF16 = mybir.dt.bfloat16
OP = mybir.AluOpType
ACT = mybir.ActivationFunctionType

B, N = 2048, 8192
NCORES = 8
RPC = B // NCORES          # rows per core
PB = 128                   # partition block (rows per job)
HALO_L = 2                 # exact reach of the closed-form forward pass
HALO_R = 12                # backward-scan warmup (contraction <= 0.77/step)

# minimax fit alpha^3 ~= P3*(alpha+H3)^2 + R3 on [0, 0.3), max err 8.44e-4
P3 = 0.45
H3 = -0.05625
R3 = -0.00058007812
SQP = float(np.sqrt(P3))            # Q = Square(SQP*alpha + SQP*H3)
SQPH = float(np.float32(SQP * H3))

DEFAULT_STRIPS = (704, 1504, 1568, 1536, 1536, 1344)


def build_core_program(nc, rows=RPC, n=N, strips=DEFAULT_STRIPS,
                       halo_l=HALO_L, halo_r=HALO_R, bufs=8,
                       eng_g=("split", 0.25), eng_q="dve", eng_t="dve",
                       eng_u="pool",
                       nr_mode="sub", w_mode="2t",
                       c_mode="act", m3_mode="lin",
                       lags=(1, 1, 3, 3, 4, 5), fb_chunks=4, lat_edge=(1, 2)):
    assert sum(strips) == n
    alpha_d = nc.dram_tensor("alpha", [rows, n], BF16, kind="ExternalInput").ap()
    fb_d = nc.dram_tensor("fb", [PB, n], BF16, kind="ExternalInput").ap()
    out_d = nc.dram_tensor("out", [rows, n], BF16, kind="ExternalOutput").ap()

    if m3_mode not in ("lin", "actlin"):
        # bias const AP for the Q-square activation
        tb = nc.alloc_sbuf_tensor("const-q-bias", [128, 1], F32)
        nc.gpsimd.memset(tb.ap(), SQPH)
        nc.const_aps.aps[(F32, SQPH)] = tb.ap()

    n_blocks = (rows + PB - 1) // PB
    wmax = halo_l + max(strips) + halo_r

    def product(eng, out, in0, in1):
        if isinstance(eng, (list, tuple)) and eng[0] == "split":
            frac = eng[1]
            m = out.shape[1]
            k = max(2, int(m * frac)) & ~1
            nc.gpsimd.tensor_tensor(out=out[:, 0:k], in0=in0[:, 0:k],
                                    in1=in1[:, 0:k], op=OP.mult)
            nc.vector.tensor_tensor(out=out[:, k:m], in0=in0[:, k:m],
                                    in1=in1[:, k:m], op=OP.mult)
            return
        e = nc.vector if eng == "dve" else nc.gpsimd
        e.tensor_tensor(out=out, in0=in0, in1=in1, op=OP.mult)

    def pick(eng, jidx, njobs):
        if isinstance(eng, str):
            return eng
        if isinstance(eng, (list, tuple)):
            mode, k = eng
            if mode == "split":
                return eng
            if mode == "head":
                return "dve" if jidx < k else "pool"
            if mode == "tail":
                return "dve" if jidx >= njobs - k else "pool"
            raise ValueError(eng)
        k = int(round(eng * njobs))
        return "pool" if jidx < k else "dve"

    with tile.TileContext(nc) as tc:
        with tc.tile_pool(name="fixed", bufs=1) as fixed:
            fb = fixed.tile([PB, n], BF16, tag="fb", name="t_fb")
            fb_pieces = [(ci * n // fb_chunks, (ci + 1) * n // fb_chunks)
                         for ci in range(fb_chunks)]

            perblk = []
            for blk in range(n_blocks):
                order = strips if blk % 2 == 0 else strips[::-1]
                pos = 0
                row = []
                for ssz in order:
                    row.append((blk * PB, pos, ssz))
                    pos += ssz
                perblk.append(row)
            jobs = [j for pair in zip(*perblk) for j in pair]

            def front(pool, r0, s, ssz, jidx, njobs):
                """alpha DMA, ACT squares, C, m3, g."""
                w = halo_l + ssz + halo_r
                dom_lo = max(0, min(s - halo_l, n - w))
                j = {
                    "r0": r0, "s": s, "oo": s - dom_lo, "w": w, "ssz": ssz,
                    "jidx": jidx, "njobs": njobs,
                    # padded tiles: reserved zero cols for shifted reads
                    "at": pool.tile([PB, wmax + 2], BF16, tag="at", name="t_at"),
                    "a2": pool.tile([PB, wmax + 2], BF16, tag="a2", name="t_a2"),
                    "ct": pool.tile([PB, wmax + 2], BF16, tag="ct", name="t_ct"),
                    "qt": pool.tile([PB, wmax + 2], BF16, tag="qt", name="t_qt"),
                    "gt": pool.tile([PB, wmax + 2], BF16, tag="gt", name="t_gt"),
                    "nr": pool.tile([PB, wmax], BF16, tag="nr", name="t_nr"),
                    "tt": pool.tile([PB, wmax + 2], BF16, tag="tt", name="t_tt"),
                }
                at, a2, ct, qt = j["at"], j["a2"], j["ct"], j["qt"]
                nc.sync.dma_start(out=at[:, 0:w],
                                  in_=alpha_d[r0:r0 + PB, dom_lo:dom_lo + w])
                nc.gpsimd.memset(a2[:, 0:1], 0.0)
                if nr_mode != "sub":
                    nc.gpsimd.memset(qt[:, 0:1], 0.0)
                nc.scalar.activation(a2[:, 1:w + 1], at[:, 0:w], ACT.Square,
                                     bias=0.0, scale=1.0)
                nc.scalar.activation(qt[:, 1:w + 1], at[:, 0:w], ACT.Square,
                                     bias=SQPH, scale=SQP)
                if c_mode == "act":
                    st = j["tt"]  # stage S in tt (dead until t)
                    nc.scalar.activation(st[:, 1:w + 1], at[:, 0:w], ACT.Square,
                                         bias=1.0, scale=1.0)
                    nc.scalar.activation(ct[:, 1:w + 1], st[:, 1:w + 1],
                                         ACT.Copy, bias=-1.0, scale=1.0)
                else:
                    nc.scalar.activation(ct[:, 1:w + 1], at[:, 0:w], ACT.Square,
                                         bias=1.0, scale=1.0)
                    nc.vector.tensor_scalar(out=ct[:, 1:w + 1], in0=ct[:, 1:w + 1],
                                            scalar1=-1.0, scalar2=None, op0=OP.add)
                if m3_mode == "act":
                    nc.scalar.activation(qt[:, 1:w + 1], qt[:, 1:w + 1], ACT.Copy,
                                         bias=R3 - 1.0, scale=1.0)
                nc.gpsimd.memset(j["at"][:, 0:1], 0.0)
                # zero the t-shift pad the y-scan reads (guards NaN garbage)
                nc.gpsimd.memset(j["tt"][:, w + 1:w + 2], 0.0)
                return j

            def st_prep(j):
                """m3 = Q+(r-1) [DVE TS] and g = A2[k-1]*C."""
                w, a2, ct, qt = j["w"], j["a2"], j["ct"], j["qt"]
                if m3_mode != "act":
                    nc.vector.tensor_scalar(out=qt[:, 1:w + 1], in0=qt[:, 1:w + 1],
                                            scalar1=R3 - 1.0, scalar2=None,
                                            op0=OP.add)
                eg = ("dve" if (j["jidx"] < 2 or j["jidx"] >= j["njobs"] - 4)
                      else pick(eng_g, j["jidx"], j["njobs"]))
                product(eg, j["gt"][:, 1:w + 1],
                        a2[:, 0:w], ct[:, 1:w + 1])

            def is_edge(j):
                return (j["jidx"] < lat_edge[0]
                        or j["jidx"] >= j["njobs"] - lat_edge[1])


            def st_nr(j):
                """nr = m3 + g*m3[-1] (2t) or forward scan."""
                w = j["w"]
                if nr_mode == "sub":
                    nc.vector.tensor_tensor(out=j["nr"][:, 0:w],
                                            in0=j["qt"][:, 1:w + 1],
                                            in1=j["gt"][:, 1:w + 1],
                                            op=OP.subtract)
                elif nr_mode == "2t" and not is_edge(j):
                    nc.vector.tensor_tensor(out=j["nr"][:, 0:w],
                                            in0=j["gt"][:, 1:w + 1],
                                            in1=j["qt"][:, 0:w], op=OP.mult)
                    nc.gpsimd.dma_start(out=j["nr"][:, 0:w],
                                        in_=j["qt"][:, 1:w + 1], accum_op=OP.add)
                else:
                    nc.vector.tensor_tensor_scan(
                        out=j["nr"][:, 0:w], data0=j["gt"][:, 1:w + 1],
                        data1=j["qt"][:, 1:w + 1],
                        initial=0.0, op0=OP.mult, op1=OP.add,
                    )

            def st_q(j):
                """q = A2*nr into gt (g dead); t = C*nr into tt."""
                w = j["w"]
                product("dve", j["gt"][:, 1:w + 1],
                        j["a2"][:, 1:w + 1], j["nr"][:, 0:w])
                product(pick(eng_t, j["jidx"], j["njobs"]), j["tt"][:, 1:w + 1],
                        j["ct"][:, 1:w + 1], j["nr"][:, 0:w])

            def st_w(j):
                """w = f + (q*f)[-1] (2t) into at (alpha dead), or scan."""
                w = j["w"]
                dom_lo = j["s"] - j["oo"]
                fbs = fb[:, dom_lo:dom_lo + w]
                nc.vector.tensor_tensor(out=j["at"][:, 1:w + 1],
                                        in0=j["gt"][:, 1:w + 1],
                                        in1=fbs, op=OP.mult)
                if not is_edge(j):
                    nc.gpsimd.dma_start(out=j["at"][:, 0:w], in_=fbs,
                                        accum_op=OP.add)
                else:
                    nc.vector.tensor_tensor(out=j["at"][:, 0:w],
                                            in0=j["at"][:, 0:w], in1=fbs,
                                            op=OP.add)

            def st_y(j):
                """backward scan: y_i = t_{i+1}*y_{i+1} - w_i, into qt."""
                w = j["w"]
                nc.vector.tensor_tensor_scan(
                    out=j["qt"][:, 0:w][:, ::-1],
                    data0=j["tt"][:, 2:w + 2][:, ::-1],
                    data1=j["at"][:, 0:w][:, ::-1],
                    initial=0.0, op0=OP.mult, op1=OP.subtract,
                )

            def st_u(j):
                """u = nr*y into ct (C dead), DMA out."""
                oo, s, r0, m = j["oo"], j["s"], j["r0"], j["ssz"]
                ut = j["ct"]
                eng = "dve" if is_edge(j) else pick(eng_u, j["jidx"], j["njobs"])
                product(eng, ut[:, 0:m],
                        j["nr"][:, oo:oo + m], j["qt"][:, oo:oo + m])
                nc.sync.dma_start(out=out_d[r0:r0 + PB, s:s + m], in_=ut[:, 0:m])

            stages = [st_prep, st_nr, st_q, st_w, st_y, st_u]
            with tc.tile_pool(name="jobs", bufs=bufs) as pool:
                live = []
                nj = len(jobs)
                pieces = list(fb_pieces)
                for k in range(nj + max(lags)):
                    if k < nj:
                        r0, s, ssz = jobs[k]
                        live.append(front(pool, r0, s, ssz, k, nj))
                    if pieces and k >= 1:
                        lo, hi = pieces.pop(0)
                        nc.sync.dma_start(out=fb[:, lo:hi], in_=fb_d[:, lo:hi])
                    for fn, lag in zip(stages, lags):
                        i = k - lag
                        if 0 <= i < nj:
                            fn(live[i])
    return nc


_cached = None


def _get_program():
    global _cached
    if _cached is None:
        nc = bacc.Bacc("TRN2", target_bir_lowering=False, debug=False)
        build_core_program(nc)
        nc.compile()
        _cached = nc
    return _cached


def _in_maps(alpha, f):
    alpha16 = np.ascontiguousarray(alpha.astype(bfloat16))
    fb = np.ascontiguousarray(
        np.broadcast_to(f.astype(bfloat16).reshape(1, N), (PB, N))
    )
    return [
        {"alpha": alpha16[c * RPC:(c + 1) * RPC], "fb": fb}
        for c in range(NCORES)
    ]


def kernel(alpha: np.ndarray, f: np.ndarray) -> np.ndarray:
    alpha = np.ascontiguousarray(alpha, dtype=np.float32)
    f = np.ascontiguousarray(f, dtype=np.float32)
    nc = _get_program()
    res = bass_utils.run_bass_kernel_spmd(nc, _in_maps(alpha, f),
                                          core_ids=list(range(NCORES)))
    out = np.concatenate([r["out"] for r in res.results], axis=0)
    return out.astype(np.float32)


if __name__ == "__main__":
    rng = np.random.default_rng(0)
    a = (0.3 * rng.random((B, N))).astype(np.float32)
    fv = rng.standard_normal(N).astype(np.float32)
    u = kernel(a, fv)
    print(u.shape, u.dtype, np.abs(u).max())


# revision 34
# speedup vs baseline: 1.3516x; 1.0019x over previous
"""Batched tridiagonal (Thomas) solve on 8 TRN2 NeuronCores.

System per row (alpha in [0, 0.3)):
    sub a_i = alpha_{i-1}^2, diag b_i = 1 + alpha_i^3,
    super c_i = CS_{i+1},  CS_j = alpha_j^2 + 2 alpha_j

Forward elimination is contraction-dominated (|g| <= 0.097, |q| <= 0.11
per step), so both forward recurrences collapse to closed forms
(numerically validated: end-to-end rel err ~7e-3 vs the 2e-2 budget):
    nr_i ~= m3_i - g_i                     (nr ~= -1/denom; 1/x ~= 2-x,
                                            m3 = b-2 via minimax-linear a^3)
    w_i  ~= f_i + (q*f)_{i-1}              (dp numerator, 2-term Neumann)
Only the backward substitution (decay 0.77/step) runs as a real
tensor_tensor_scan:  y_i = t_{i+1}*y_{i+1} - w_i,  u = nr*y.

Engine split per (128-row, strip) job, all bf16:
  ACT : A2 = a^2, S = (a+1)^2, C = Copy(S-1)
  DVE : m3 = L1*a+(L0-1) [TS], nr = m3-g [TT], products g/q/t1w/t
        [bf16 2x TT], the backward y-scan, edge-job forward scans
  Pool: u product, a column-split share of g, SWDGE issue of the w add
  DMA : alpha in, u out, shared-f broadcast, and the w-assembly "+f"
        via an accum-add DMA (dst += in) on the otherwise idle DMA fleet.
The first/last jobs ("edge") use true forward scans and DVE-only paths
to minimize pipeline fill/drain latency; interior jobs use the
throughput path above, software-pipelined via staged lags.

Sharding: pure data parallel over batch rows (256 rows/core = 2 blocks
of 128 partitions); columns split into strips with contraction halos so
every job is independent. f is shared: one bf16 [128, 8192] broadcast
load per core. Host does dtype casts and the final fp32 cast.
"""

import sys

sys.path.insert(0, "/opt/trn_rl_repo")

import numpy as np
from ml_dtypes import bfloat16

from concourse import bacc, mybir, tile
from concourse import bass_utils

F32 = mybir.dt.float32
BF16 = mybir.dt.bfloat16
OP = mybir.AluOpType
ACT = mybir.ActivationFunctionType

B, N = 2048, 8192
NCORES = 8
RPC = B // NCORES          # rows per core
PB = 128                   # partition block (rows per job)
HALO_L = 2                 # exact reach of the closed-form forward pass
HALO_R = 8                 # backward-scan warmup (contraction <= 0.77/step)

# minimax fit alpha^3 ~= P3*(alpha+H3)^2 + R3 on [0, 0.3), max err 8.44e-4
P3 = 0.45
H3 = -0.05625
R3 = -0.00058007812
SQP = float(np.sqrt(P3))            # Q = Square(SQP*alpha + SQP*H3)
SQPH = float(np.float32(SQP * H3))

DEFAULT_STRIPS = (704, 1504, 1568, 1536, 1536, 1344)


def build_core_program(nc, rows=RPC, n=N, strips=DEFAULT_STRIPS,
                       halo_l=HALO_L, halo_r=HALO_R, bufs=8,
                       eng_g=("split", 0.25), eng_q="dve", eng_t="dve",
                       eng_u="pool",
                       nr_mode="sub", w_mode="2t",
                       c_mode="act", m3_mode="lin",
                       lags=(1, 1, 3, 3, 4, 5), fb_chunks=4, lat_edge=(1, 2)):
    assert sum(strips) == n
    alpha_d = nc.dram_tensor("alpha", [rows, n], BF16, kind="ExternalInput").ap()
    fb_d = nc.dram_tensor("fb", [PB, n], BF16, kind="ExternalInput").ap()
    out_d = nc.dram_tensor("out", [rows, n], BF16, kind="ExternalOutput").ap()

    if m3_mode not in ("lin", "actlin"):
        # bias const AP for the Q-square activation
        tb = nc.alloc_sbuf_tensor("const-q-bias", [128, 1], F32)
        nc.gpsimd.memset(tb.ap(), SQPH)
        nc.const_aps.aps[(F32, SQPH)] = tb.ap()

    n_blocks = (rows + PB - 1) // PB
    wmax = halo_l + max(strips) + halo_r

    def product(eng, out, in0, in1):
        if isinstance(eng, (list, tuple)) and eng[0] == "split":
            frac = eng[1]
            m = out.shape[1]
            k = max(2, int(m * frac)) & ~1
            nc.gpsimd.tensor_tensor(out=out[:, 0:k], in0=in0[:, 0:k],
                                    in1=in1[:, 0:k], op=OP.mult)
            nc.vector.tensor_tensor(out=out[:, k:m], in0=in0[:, k:m],
                                    in1=in1[:, k:m], op=OP.mult)
            return
        e = nc.vector if eng == "dve" else nc.gpsimd
        e.tensor_tensor(out=out, in0=in0, in1=in1, op=OP.mult)

    def pick(eng, jidx, njobs):
        if isinstance(eng, str):
            return eng
        if isinstance(eng, (list, tuple)):
            mode, k = eng
            if mode == "split":
                return eng
            if mode == "head":
                return "dve" if jidx < k else "pool"
            if mode == "tail":
                return "dve" if jidx >= njobs - k else "pool"
            raise ValueError(eng)
        k = int(round(eng * njobs))
        return "pool" if jidx < k else "dve"

    with tile.TileContext(nc) as tc:
        with tc.tile_pool(name="fixed", bufs=1) as fixed:
            fb = fixed.tile([PB, n], BF16, tag="fb", name="t_fb")
            fb_pieces = [(ci * n // fb_chunks, (ci + 1) * n // fb_chunks)
                         for ci in range(fb_chunks)]

            perblk = []
            for blk in range(n_blocks):
                order = strips if blk % 2 == 0 else strips[::-1]
                pos = 0
                row = []
                for ssz in order:
                    row.append((blk * PB, pos, ssz))
                    pos += ssz
                perblk.append(row)
            jobs = [j for pair in zip(*perblk) for j in pair]

            def front(pool, r0, s, ssz, jidx, njobs):
                """alpha DMA, ACT squares, C, m3, g."""
                w = halo_l + ssz + halo_r
                dom_lo = max(0, min(s - halo_l, n - w))
                j = {
                    "r0": r0, "s": s, "oo": s - dom_lo, "w": w, "ssz": ssz,
                    "jidx": jidx, "njobs": njobs,
                    # padded tiles: reserved zero cols for shifted reads
                    "at": pool.tile([PB, wmax + 2], BF16, tag="at", name="t_at"),
                    "a2": pool.tile([PB, wmax + 2], BF16, tag="a2", name="t_a2"),
                    "ct": pool.tile([PB, wmax + 2], BF16, tag="ct", name="t_ct"),
                    "qt": pool.tile([PB, wmax + 2], BF16, tag="qt", name="t_qt"),
                    "gt": pool.tile([PB, wmax + 2], BF16, tag="gt", name="t_gt"),
                    "nr": pool.tile([PB, wmax], BF16, tag="nr", name="t_nr"),
                    "tt": pool.tile([PB, wmax + 2], BF16, tag="tt", name="t_tt"),
                }
                at, a2, ct, qt = j["at"], j["a2"], j["ct"], j["qt"]
                nc.sync.dma_start(out=at[:, 0:w],
                                  in_=alpha_d[r0:r0 + PB, dom_lo:dom_lo + w])
                nc.gpsimd.memset(a2[:, 0:1], 0.0)
                if nr_mode != "sub":
                    nc.gpsimd.memset(qt[:, 0:1], 0.0)
                nc.scalar.activation(a2[:, 1:w + 1], at[:, 0:w], ACT.Square,
                                     bias=0.0, scale=1.0)
                nc.scalar.activation(qt[:, 1:w + 1], at[:, 0:w], ACT.Square,
                                     bias=SQPH, scale=SQP)
                if c_mode == "act":
                    st = j["tt"]  # stage S in tt (dead until t)
                    nc.scalar.activation(st[:, 1:w + 1], at[:, 0:w], ACT.Square,
                                         bias=1.0, scale=1.0)
                    nc.scalar.activation(ct[:, 1:w + 1], st[:, 1:w + 1],
                                         ACT.Copy, bias=-1.0, scale=1.0)
                else:
                    nc.scalar.activation(ct[:, 1:w + 1], at[:, 0:w], ACT.Square,
                                         bias=1.0, scale=1.0)
                    nc.vector.tensor_scalar(out=ct[:, 1:w + 1], in0=ct[:, 1:w + 1],
                                            scalar1=-1.0, scalar2=None, op0=OP.add)
                if m3_mode == "act":
                    nc.scalar.activation(qt[:, 1:w + 1], qt[:, 1:w + 1], ACT.Copy,
                                         bias=R3 - 1.0, scale=1.0)
                nc.gpsimd.memset(j["at"][:, 0:1], 0.0)
                # zero the t-shift pad the y-scan reads (guards NaN garbage)
                nc.gpsimd.memset(j["tt"][:, w + 1:w + 2], 0.0)
                return j

            def st_prep(j):
                """m3 = Q+(r-1) [DVE TS] and g = A2[k-1]*C."""
                w, a2, ct, qt = j["w"], j["a2"], j["ct"], j["qt"]
                if m3_mode != "act":
                    nc.vector.tensor_scalar(out=qt[:, 1:w + 1], in0=qt[:, 1:w + 1],
                                            scalar1=R3 - 1.0, scalar2=None,
                                            op0=OP.add)
                eg = ("dve" if (j["jidx"] < 2 or j["jidx"] >= j["njobs"] - 4)
                      else pick(eng_g, j["jidx"], j["njobs"]))
                product(eg, j["gt"][:, 1:w + 1],
                        a2[:, 0:w], ct[:, 1:w + 1])

            def is_edge(j):
                return (j["jidx"] < lat_edge[0]
                        or j["jidx"] >= j["njobs"] - lat_edge[1])


            def st_nr(j):
                """nr = m3 + g*m3[-1] (2t) or forward scan."""
                w = j["w"]
                if nr_mode == "sub":
                    nc.vector.tensor_tensor(out=j["nr"][:, 0:w],
                                            in0=j["qt"][:, 1:w + 1],
                                            in1=j["gt"][:, 1:w + 1],
                                            op=OP.subtract)
                elif nr_mode == "2t" and not is_edge(j):
                    nc.vector.tensor_tensor(out=j["nr"][:, 0:w],
                                            in0=j["gt"][:, 1:w + 1],
                                            in1=j["qt"][:, 0:w], op=OP.mult)
                    nc.gpsimd.dma_start(out=j["nr"][:, 0:w],
                                        in_=j["qt"][:, 1:w + 1], accum_op=OP.add)
                else:
                    nc.vector.tensor_tensor_scan(
                        out=j["nr"][:, 0:w], data0=j["gt"][:, 1:w + 1],
                        data1=j["qt"][:, 1:w + 1],
                        initial=0.0, op0=OP.mult, op1=OP.add,
                    )

            def st_q(j):
                """q = A2*nr into gt (g dead); t = C*nr into tt."""
                w = j["w"]
                product("dve", j["gt"][:, 1:w + 1],
                        j["a2"][:, 1:w + 1], j["nr"][:, 0:w])
                product(pick(eng_t, j["jidx"], j["njobs"]), j["tt"][:, 1:w + 1],
                        j["ct"][:, 1:w + 1], j["nr"][:, 0:w])

            def st_w(j):
                """w = f + (q*f)[-1] (2t) into at (alpha dead), or scan."""
                w = j["w"]
                dom_lo = j["s"] - j["oo"]
                fbs = fb[:, dom_lo:dom_lo + w]
                nc.vector.tensor_tensor(out=j["at"][:, 1:w + 1],
                                        in0=j["gt"][:, 1:w + 1],
                                        in1=fbs, op=OP.mult)
                if not is_edge(j):
                    nc.gpsimd.dma_start(out=j["at"][:, 0:w], in_=fbs,
                                        accum_op=OP.add)
                else:
                    nc.vector.tensor_tensor(out=j["at"][:, 0:w],
                                            in0=j["at"][:, 0:w], in1=fbs,
                                            op=OP.add)

            def st_y(j):
                """backward scan: y_i = t_{i+1}*y_{i+1} - w_i, into qt."""
                w = j["w"]
                nc.vector.tensor_tensor_scan(
                    out=j["qt"][:, 0:w][:, ::-1],
                    data0=j["tt"][:, 2:w + 2][:, ::-1],
                    data1=j["at"][:, 0:w][:, ::-1],
                    initial=0.0, op0=OP.mult, op1=OP.subtract,
                )

            def st_u(j):
                """u = nr*y into ct (C dead), DMA out."""
                oo, s, r0, m = j["oo"], j["s"], j["r0"], j["ssz"]
                ut = j["ct"]
                eng = "dve" if is_edge(j) else pick(eng_u, j["jidx"], j["njobs"])
                product(eng, ut[:, 0:m],
                        j["nr"][:, oo:oo + m], j["qt"][:, oo:oo + m])
                nc.sync.dma_start(out=out_d[r0:r0 + PB, s:s + m], in_=ut[:, 0:m])

            stages = [st_prep, st_nr, st_q, st_w, st_y, st_u]
            with tc.tile_pool(name="jobs", bufs=bufs) as pool:
                live = []
                nj = len(jobs)
                pieces = list(fb_pieces)
                for k in range(nj + max(lags)):
                    if k < nj:
                        r0, s, ssz = jobs[k]
                        live.append(front(pool, r0, s, ssz, k, nj))
                    if pieces and k >= 1:
                        lo, hi = pieces.pop(0)
                        nc.sync.dma_start(out=fb[:, lo:hi], in_=fb_d[:, lo:hi])
                    for fn, lag in zip(stages, lags):
                        i = k - lag
                        if 0 <= i < nj:
                            fn(live[i])
    return nc


_cached = None


def _get_program():
    global _cached
    if _cached is None:
        nc = bacc.Bacc("TRN2", target_bir_lowering=False, debug=False)
        build_core_program(nc)
        nc.compile()
        _cached = nc
    return _cached


def _in_maps(alpha, f):
    alpha16 = np.ascontiguousarray(alpha.astype(bfloat16))
    fb = np.ascontiguousarray(
        np.broadcast_to(f.astype(bfloat16).reshape(1, N), (PB, N))
    )
    return [
        {"alpha": alpha16[c * RPC:(c + 1) * RPC], "fb": fb}
        for c in range(NCORES)
    ]


def kernel(alpha: np.ndarray, f: np.ndarray) -> np.ndarray:
    alpha = np.ascontiguousarray(alpha, dtype=np.float32)
    f = np.ascontiguousarray(f, dtype=np.float32)
    nc = _get_program()
    res = bass_utils.run_bass_kernel_spmd(nc, _in_maps(alpha, f),
                                          core_ids=list(range(NCORES)))
    out = np.concatenate([r["out"] for r in res.results], axis=0)
    return out.astype(np.float32)


if __name__ == "__main__":
    rng = np.random.default_rng(0)
    a = (0.3 * rng.random((B, N))).astype(np.float32)
    fv = rng.standard_normal(N).astype(np.float32)
    u = kernel(a, fv)
    print(u.shape, u.dtype, np.abs(u).max())


# revision 35
# speedup vs baseline: 1.3529x; 1.0010x over previous
"""Batched tridiagonal (Thomas) solve on 8 TRN2 NeuronCores.

System per row (alpha in [0, 0.3)):
    sub a_i = alpha_{i-1}^2, diag b_i = 1 + alpha_i^3,
    super c_i = CS_{i+1},  CS_j = alpha_j^2 + 2 alpha_j

Forward elimination is contraction-dominated (|g| <= 0.097, |q| <= 0.11
per step), so both forward recurrences collapse to closed forms
(numerically validated: end-to-end rel err ~7e-3 vs the 2e-2 budget):
    nr_i ~= m3_i - g_i                     (nr ~= -1/denom; 1/x ~= 2-x,
                                            m3 = b-2 via minimax-linear a^3)
    w_i  ~= f_i + (q*f)_{i-1}              (dp numerator, 2-term Neumann)
Only the backward substitution (decay 0.77/step) runs as a real
tensor_tensor_scan:  y_i = t_{i+1}*y_{i+1} - w_i,  u = nr*y.

Engine split per (128-row, strip) job, all bf16:
  ACT : A2 = a^2, S = (a+1)^2, C = Copy(S-1)
  DVE : m3 = L1*a+(L0-1) [TS], nr = m3-g [TT], products g/q/t1w/t
        [bf16 2x TT], the backward y-scan, edge-job forward scans
  Pool: u product, a column-split share of g, SWDGE issue of the w add
  DMA : alpha in, u out, shared-f broadcast, and the w-assembly "+f"
        via an accum-add DMA (dst += in) on the otherwise idle DMA fleet.
The first/last jobs ("edge") use true forward scans and DVE-only paths
to minimize pipeline fill/drain latency; interior jobs use the
throughput path above, software-pipelined via staged lags.

Sharding: pure data parallel over batch rows (256 rows/core = 2 blocks
of 128 partitions); columns split into strips with contraction halos so
every job is independent. f is shared: one bf16 [128, 8192] broadcast
load per core. Host does dtype casts and the final fp32 cast.
"""

import sys

sys.path.insert(0, "/opt/trn_rl_repo")

import numpy as np
from ml_dtypes import bfloat16

from concourse import bacc, mybir, tile
from concourse import bass_utils

F32 = mybir.dt.float32
BF16 = mybir.dt.bfloat16
OP = mybir.AluOpType
ACT = mybir.ActivationFunctionType

B, N = 2048, 8192
NCORES = 8
RPC = B // NCORES          # rows per core
PB = 128                   # partition block (rows per job)
HALO_L = 2                 # exact reach of the closed-form forward pass
HALO_R = 6                 # backward-scan warmup (contraction <= 0.77/step)

# minimax fit alpha^3 ~= P3*(alpha+H3)^2 + R3 on [0, 0.3), max err 8.44e-4
P3 = 0.45
H3 = -0.05625
R3 = -0.00058007812
SQP = float(np.sqrt(P3))            # Q = Square(SQP*alpha + SQP*H3)
SQPH = float(np.float32(SQP * H3))

DEFAULT_STRIPS = (704, 1504, 1568, 1536, 1536, 1344)


def build_core_program(nc, rows=RPC, n=N, strips=DEFAULT_STRIPS,
                       halo_l=HALO_L, halo_r=HALO_R, bufs=8,
                       eng_g=("split", 0.25), eng_q="dve", eng_t="dve",
                       eng_u="pool",
                       nr_mode="sub", w_mode="2t",
                       c_mode="act", m3_mode="lin",
                       lags=(1, 1, 3, 3, 4, 5), fb_chunks=4, lat_edge=(1, 2)):
    assert sum(strips) == n
    alpha_d = nc.dram_tensor("alpha", [rows, n], BF16, kind="ExternalInput").ap()
    fb_d = nc.dram_tensor("fb", [PB, n], BF16, kind="ExternalInput").ap()
    out_d = nc.dram_tensor("out", [rows, n], BF16, kind="ExternalOutput").ap()

    if m3_mode not in ("lin", "actlin"):
        # bias const AP for the Q-square activation
        tb = nc.alloc_sbuf_tensor("const-q-bias", [128, 1], F32)
        nc.gpsimd.memset(tb.ap(), SQPH)
        nc.const_aps.aps[(F32, SQPH)] = tb.ap()

    n_blocks = (rows + PB - 1) // PB
    wmax = halo_l + max(strips) + halo_r

    def product(eng, out, in0, in1):
        if isinstance(eng, (list, tuple)) and eng[0] == "split":
            frac = eng[1]
            m = out.shape[1]
            k = max(2, int(m * frac)) & ~1
            nc.gpsimd.tensor_tensor(out=out[:, 0:k], in0=in0[:, 0:k],
                                    in1=in1[:, 0:k], op=OP.mult)
            nc.vector.tensor_tensor(out=out[:, k:m], in0=in0[:, k:m],
                                    in1=in1[:, k:m], op=OP.mult)
            return
        e = nc.vector if eng == "dve" else nc.gpsimd
        e.tensor_tensor(out=out, in0=in0, in1=in1, op=OP.mult)

    def pick(eng, jidx, njobs):
        if isinstance(eng, str):
            return eng
        if isinstance(eng, (list, tuple)):
            mode, k = eng
            if mode == "split":
                return eng
            if mode == "head":
                return "dve" if jidx < k else "pool"
            if mode == "tail":
                return "dve" if jidx >= njobs - k else "pool"
            raise ValueError(eng)
        k = int(round(eng * njobs))
        return "pool" if jidx < k else "dve"

    with tile.TileContext(nc) as tc:
        with tc.tile_pool(name="fixed", bufs=1) as fixed:
            fb = fixed.tile([PB, n], BF16, tag="fb", name="t_fb")
            fb_pieces = [(ci * n // fb_chunks, (ci + 1) * n // fb_chunks)
                         for ci in range(fb_chunks)]

            perblk = []
            for blk in range(n_blocks):
                order = strips if blk % 2 == 0 else strips[::-1]
                pos = 0
                row = []
                for ssz in order:
                    row.append((blk * PB, pos, ssz))
                    pos += ssz
                perblk.append(row)
            jobs = [j for pair in zip(*perblk) for j in pair]

            def front(pool, r0, s, ssz, jidx, njobs):
                """alpha DMA, ACT squares, C, m3, g."""
                w = halo_l + ssz + halo_r
                dom_lo = max(0, min(s - halo_l, n - w))
                j = {
                    "r0": r0, "s": s, "oo": s - dom_lo, "w": w, "ssz": ssz,
                    "jidx": jidx, "njobs": njobs,
                    # padded tiles: reserved zero cols for shifted reads
                    "at": pool.tile([PB, wmax + 2], BF16, tag="at", name="t_at"),
                    "a2": pool.tile([PB, wmax + 2], BF16, tag="a2", name="t_a2"),
                    "ct": pool.tile([PB, wmax + 2], BF16, tag="ct", name="t_ct"),
                    "qt": pool.tile([PB, wmax + 2], BF16, tag="qt", name="t_qt"),
                    "gt": pool.tile([PB, wmax + 2], BF16, tag="gt", name="t_gt"),
                    "nr": pool.tile([PB, wmax], BF16, tag="nr", name="t_nr"),
                    "tt": pool.tile([PB, wmax + 2], BF16, tag="tt", name="t_tt"),
                }
                at, a2, ct, qt = j["at"], j["a2"], j["ct"], j["qt"]
                nc.sync.dma_start(out=at[:, 0:w],
                                  in_=alpha_d[r0:r0 + PB, dom_lo:dom_lo + w])
                nc.gpsimd.memset(a2[:, 0:1], 0.0)
                if nr_mode != "sub":
                    nc.gpsimd.memset(qt[:, 0:1], 0.0)
                nc.scalar.activation(a2[:, 1:w + 1], at[:, 0:w], ACT.Square,
                                     bias=0.0, scale=1.0)
                nc.scalar.activation(qt[:, 1:w + 1], at[:, 0:w], ACT.Square,
                                     bias=SQPH, scale=SQP)
                if c_mode == "act":
                    st = j["tt"]  # stage S in tt (dead until t)
                    nc.scalar.activation(st[:, 1:w + 1], at[:, 0:w], ACT.Square,
                                         bias=1.0, scale=1.0)
                    nc.scalar.activation(ct[:, 1:w + 1], st[:, 1:w + 1],
                                         ACT.Copy, bias=-1.0, scale=1.0)
                else:
                    nc.scalar.activation(ct[:, 1:w + 1], at[:, 0:w], ACT.Square,
                                         bias=1.0, scale=1.0)
                    nc.vector.tensor_scalar(out=ct[:, 1:w + 1], in0=ct[:, 1:w + 1],
                                            scalar1=-1.0, scalar2=None, op0=OP.add)
                if m3_mode == "act":
                    nc.scalar.activation(qt[:, 1:w + 1], qt[:, 1:w + 1], ACT.Copy,
                                         bias=R3 - 1.0, scale=1.0)
                nc.gpsimd.memset(j["at"][:, 0:1], 0.0)
                # zero the t-shift pad the y-scan reads (guards NaN garbage)
                nc.gpsimd.memset(j["tt"][:, w + 1:w + 2], 0.0)
                return j

            def st_prep(j):
                """m3 = Q+(r-1) [DVE TS] and g = A2[k-1]*C."""
                w, a2, ct, qt = j["w"], j["a2"], j["ct"], j["qt"]
                if m3_mode != "act":
                    nc.vector.tensor_scalar(out=qt[:, 1:w + 1], in0=qt[:, 1:w + 1],
                                            scalar1=R3 - 1.0, scalar2=None,
                                            op0=OP.add)
                eg = ("dve" if (j["jidx"] < 2 or j["jidx"] >= j["njobs"] - 4)
                      else pick(eng_g, j["jidx"], j["njobs"]))
                product(eg, j["gt"][:, 1:w + 1],
                        a2[:, 0:w], ct[:, 1:w + 1])

            def is_edge(j):
                return (j["jidx"] < lat_edge[0]
                        or j["jidx"] >= j["njobs"] - lat_edge[1])


            def st_nr(j):
                """nr = m3 + g*m3[-1] (2t) or forward scan."""
                w = j["w"]
                if nr_mode == "sub":
                    nc.vector.tensor_tensor(out=j["nr"][:, 0:w],
                                            in0=j["qt"][:, 1:w + 1],
                                            in1=j["gt"][:, 1:w + 1],
                                            op=OP.subtract)
                elif nr_mode == "2t" and not is_edge(j):
                    nc.vector.tensor_tensor(out=j["nr"][:, 0:w],
                                            in0=j["gt"][:, 1:w + 1],
                                            in1=j["qt"][:, 0:w], op=OP.mult)
                    nc.gpsimd.dma_start(out=j["nr"][:, 0:w],
                                        in_=j["qt"][:, 1:w + 1], accum_op=OP.add)
                else:
                    nc.vector.tensor_tensor_scan(
                        out=j["nr"][:, 0:w], data0=j["gt"][:, 1:w + 1],
                        data1=j["qt"][:, 1:w + 1],
                        initial=0.0, op0=OP.mult, op1=OP.add,
                    )

            def st_q(j):
                """q = A2*nr into gt (g dead); t = C*nr into tt."""
                w = j["w"]
                product("dve", j["gt"][:, 1:w + 1],
                        j["a2"][:, 1:w + 1], j["nr"][:, 0:w])
                product(pick(eng_t, j["jidx"], j["njobs"]), j["tt"][:, 1:w + 1],
                        j["ct"][:, 1:w + 1], j["nr"][:, 0:w])

            def st_w(j):
                """w = f + (q*f)[-1] (2t) into at (alpha dead), or scan."""
                w = j["w"]
                dom_lo = j["s"] - j["oo"]
                fbs = fb[:, dom_lo:dom_lo + w]
                nc.vector.tensor_tensor(out=j["at"][:, 1:w + 1],
                                        in0=j["gt"][:, 1:w + 1],
                                        in1=fbs, op=OP.mult)
                if not is_edge(j):
                    nc.gpsimd.dma_start(out=j["at"][:, 0:w], in_=fbs,
                                        accum_op=OP.add)
                else:
                    nc.vector.tensor_tensor(out=j["at"][:, 0:w],
                                            in0=j["at"][:, 0:w], in1=fbs,
                                            op=OP.add)

            def st_y(j):
                """backward scan: y_i = t_{i+1}*y_{i+1} - w_i, into qt."""
                w = j["w"]
                nc.vector.tensor_tensor_scan(
                    out=j["qt"][:, 0:w][:, ::-1],
                    data0=j["tt"][:, 2:w + 2][:, ::-1],
                    data1=j["at"][:, 0:w][:, ::-1],
                    initial=0.0, op0=OP.mult, op1=OP.subtract,
                )

            def st_u(j):
                """u = nr*y into ct (C dead), DMA out."""
                oo, s, r0, m = j["oo"], j["s"], j["r0"], j["ssz"]
                ut = j["ct"]
                eng = "dve" if is_edge(j) else pick(eng_u, j["jidx"], j["njobs"])
                product(eng, ut[:, 0:m],
                        j["nr"][:, oo:oo + m], j["qt"][:, oo:oo + m])
                nc.sync.dma_start(out=out_d[r0:r0 + PB, s:s + m], in_=ut[:, 0:m])

            stages = [st_prep, st_nr, st_q, st_w, st_y, st_u]
            with tc.tile_pool(name="jobs", bufs=bufs) as pool:
                live = []
                nj = len(jobs)
                pieces = list(fb_pieces)
                for k in range(nj + max(lags)):
                    if k < nj:
                        r0, s, ssz = jobs[k]
                        live.append(front(pool, r0, s, ssz, k, nj))
                    if pieces and k >= 1:
                        lo, hi = pieces.pop(0)
                        nc.sync.dma_start(out=fb[:, lo:hi], in_=fb_d[:, lo:hi])
                    for fn, lag in zip(stages, lags):
                        i = k - lag
                        if 0 <= i < nj:
                            fn(live[i])
    return nc


_cached = None


def _get_program():
    global _cached
    if _cached is None:
        nc = bacc.Bacc("TRN2", target_bir_lowering=False, debug=False)
        build_core_program(nc)
        nc.compile()
        _cached = nc
    return _cached


def _in_maps(alpha, f):
    alpha16 = np.ascontiguousarray(alpha.astype(bfloat16))
    fb = np.ascontiguousarray(
        np.broadcast_to(f.astype(bfloat16).reshape(1, N), (PB, N))
    )
    return [
        {"alpha": alpha16[c * RPC:(c + 1) * RPC], "fb": fb}
        for c in range(NCORES)
    ]


def kernel(alpha: np.ndarray, f: np.ndarray) -> np.ndarray:
    alpha = np.ascontiguousarray(alpha, dtype=np.float32)
    f = np.ascontiguousarray(f, dtype=np.float32)
    nc = _get_program()
    res = bass_utils.run_bass_kernel_spmd(nc, _in_maps(alpha, f),
                                          core_ids=list(range(NCORES)))
    out = np.concatenate([r["out"] for r in res.results], axis=0)
    return out.astype(np.float32)


if __name__ == "__main__":
    rng = np.random.default_rng(0)
    a = (0.3 * rng.random((B, N))).astype(np.float32)
    fv = rng.standard_normal(N).astype(np.float32)
    u = kernel(a, fv)
    print(u.shape, u.dtype, np.abs(u).max())


# revision 36
# speedup vs baseline: 1.3562x; 1.0024x over previous
"""Batched tridiagonal (Thomas) solve on 8 TRN2 NeuronCores.

System per row (alpha in [0, 0.3)):
    sub a_i = alpha_{i-1}^2, diag b_i = 1 + alpha_i^3,
    super c_i = CS_{i+1},  CS_j = alpha_j^2 + 2 alpha_j

Forward elimination is contraction-dominated (|g| <= 0.097, |q| <= 0.11
per step), so both forward recurrences collapse to closed forms
(numerically validated: end-to-end rel err ~7e-3 vs the 2e-2 budget):
    nr_i ~= m3_i - g_i                     (nr ~= -1/denom; 1/x ~= 2-x,
                                            m3 = b-2 via minimax-linear a^3)
    w_i  ~= f_i + (q*f)_{i-1}              (dp numerator, 2-term Neumann)
Only the backward substitution (decay 0.77/step) runs as a real
tensor_tensor_scan:  y_i = t_{i+1}*y_{i+1} - w_i,  u = nr*y.

Engine split per (128-row, strip) job, all bf16:
  ACT : A2 = a^2, S = (a+1)^2, C = Copy(S-1)
  DVE : m3 = L1*a+(L0-1) [TS], nr = m3-g [TT], products g/q/t1w/t
        [bf16 2x TT], the backward y-scan, edge-job forward scans
  Pool: u product, a column-split share of g, SWDGE issue of the w add
  DMA : alpha in, u out, shared-f broadcast, and the w-assembly "+f"
        via an accum-add DMA (dst += in) on the otherwise idle DMA fleet.
The first/last jobs ("edge") use true forward scans and DVE-only paths
to minimize pipeline fill/drain latency; interior jobs use the
throughput path above, software-pipelined via staged lags.

Sharding: pure data parallel over batch rows (256 rows/core = 2 blocks
of 128 partitions); columns split into strips with contraction halos so
every job is independent. f is shared: one bf16 [128, 8192] broadcast
load per core. Host does dtype casts and the final fp32 cast.
"""

import sys

sys.path.insert(0, "/opt/trn_rl_repo")

import numpy as np
from ml_dtypes import bfloat16

from concourse import bacc, mybir, tile
from concourse import bass_utils

F32 = mybir.dt.float32
BF16 = mybir.dt.bfloat16
OP = mybir.AluOpType
ACT = mybir.ActivationFunctionType

B, N = 2048, 8192
NCORES = 8
RPC = B // NCORES          # rows per core
PB = 128                   # partition block (rows per job)
HALO_L = 2                 # exact reach of the closed-form forward pass
HALO_R = 6                 # backward-scan warmup (contraction <= 0.77/step)

# minimax fit alpha^3 ~= P3*(alpha+H3)^2 + R3 on [0, 0.3), max err 8.44e-4
P3 = 0.45
H3 = -0.05625
R3 = -0.00058007812
SQP = float(np.sqrt(P3))            # Q = Square(SQP*alpha + SQP*H3)
SQPH = float(np.float32(SQP * H3))

DEFAULT_STRIPS = (704, 1504, 1568, 1536, 1536, 1344)


def build_core_program(nc, rows=RPC, n=N, strips=DEFAULT_STRIPS,
                       halo_l=HALO_L, halo_r=HALO_R, bufs=8,
                       eng_g=("split", 0.25), eng_q="dve", eng_t="dve",
                       eng_u="pool",
                       nr_mode="sub", w_mode="2t",
                       c_mode="act", m3_mode="lin",
                       lags=(1, 1, 3, 3, 4, 5), fb_chunks=8, lat_edge=(1, 2)):
    assert sum(strips) == n
    alpha_d = nc.dram_tensor("alpha", [rows, n], BF16, kind="ExternalInput").ap()
    fb_d = nc.dram_tensor("fb", [PB, n], BF16, kind="ExternalInput").ap()
    out_d = nc.dram_tensor("out", [rows, n], BF16, kind="ExternalOutput").ap()

    if m3_mode not in ("lin", "actlin"):
        # bias const AP for the Q-square activation
        tb = nc.alloc_sbuf_tensor("const-q-bias", [128, 1], F32)
        nc.gpsimd.memset(tb.ap(), SQPH)
        nc.const_aps.aps[(F32, SQPH)] = tb.ap()

    n_blocks = (rows + PB - 1) // PB
    wmax = halo_l + max(strips) + halo_r

    def product(eng, out, in0, in1):
        if isinstance(eng, (list, tuple)) and eng[0] == "split":
            frac = eng[1]
            m = out.shape[1]
            k = max(2, int(m * frac)) & ~1
            nc.gpsimd.tensor_tensor(out=out[:, 0:k], in0=in0[:, 0:k],
                                    in1=in1[:, 0:k], op=OP.mult)
            nc.vector.tensor_tensor(out=out[:, k:m], in0=in0[:, k:m],
                                    in1=in1[:, k:m], op=OP.mult)
            return
        e = nc.vector if eng == "dve" else nc.gpsimd
        e.tensor_tensor(out=out, in0=in0, in1=in1, op=OP.mult)

    def pick(eng, jidx, njobs):
        if isinstance(eng, str):
            return eng
        if isinstance(eng, (list, tuple)):
            mode, k = eng
            if mode == "split":
                return eng
            if mode == "head":
                return "dve" if jidx < k else "pool"
            if mode == "tail":
                return "dve" if jidx >= njobs - k else "pool"
            raise ValueError(eng)
        k = int(round(eng * njobs))
        return "pool" if jidx < k else "dve"

    with tile.TileContext(nc) as tc:
        with tc.tile_pool(name="fixed", bufs=1) as fixed:
            fb = fixed.tile([PB, n], BF16, tag="fb", name="t_fb")
            fb_pieces = [(ci * n // fb_chunks, (ci + 1) * n // fb_chunks)
                         for ci in range(fb_chunks)]

            perblk = []
            for blk in range(n_blocks):
                order = strips if blk % 2 == 0 else strips[::-1]
                pos = 0
                row = []
                for ssz in order:
                    row.append((blk * PB, pos, ssz))
                    pos += ssz
                perblk.append(row)
            jobs = [j for pair in zip(*perblk) for j in pair]

            def front(pool, r0, s, ssz, jidx, njobs):
                """alpha DMA, ACT squares, C, m3, g."""
                w = halo_l + ssz + halo_r
                dom_lo = max(0, min(s - halo_l, n - w))
                j = {
                    "r0": r0, "s": s, "oo": s - dom_lo, "w": w, "ssz": ssz,
                    "jidx": jidx, "njobs": njobs,
                    # padded tiles: reserved zero cols for shifted reads
                    "at": pool.tile([PB, wmax + 2], BF16, tag="at", name="t_at"),
                    "a2": pool.tile([PB, wmax + 2], BF16, tag="a2", name="t_a2"),
                    "ct": pool.tile([PB, wmax + 2], BF16, tag="ct", name="t_ct"),
                    "qt": pool.tile([PB, wmax + 2], BF16, tag="qt", name="t_qt"),
                    "gt": pool.tile([PB, wmax + 2], BF16, tag="gt", name="t_gt"),
                    "nr": pool.tile([PB, wmax], BF16, tag="nr", name="t_nr"),
                    "tt": pool.tile([PB, wmax + 2], BF16, tag="tt", name="t_tt"),
                }
                at, a2, ct, qt = j["at"], j["a2"], j["ct"], j["qt"]
                nc.sync.dma_start(out=at[:, 0:w],
                                  in_=alpha_d[r0:r0 + PB, dom_lo:dom_lo + w])
                nc.gpsimd.memset(a2[:, 0:1], 0.0)
                if nr_mode != "sub":
                    nc.gpsimd.memset(qt[:, 0:1], 0.0)
                nc.scalar.activation(a2[:, 1:w + 1], at[:, 0:w], ACT.Square,
                                     bias=0.0, scale=1.0)
                nc.scalar.activation(qt[:, 1:w + 1], at[:, 0:w], ACT.Square,
                                     bias=SQPH, scale=SQP)
                if c_mode == "act":
                    st = j["tt"]  # stage S in tt (dead until t)
                    nc.scalar.activation(st[:, 1:w + 1], at[:, 0:w], ACT.Square,
                                         bias=1.0, scale=1.0)
                    nc.scalar.activation(ct[:, 1:w + 1], st[:, 1:w + 1],
                                         ACT.Copy, bias=-1.0, scale=1.0)
                else:
                    nc.scalar.activation(ct[:, 1:w + 1], at[:, 0:w], ACT.Square,
                                         bias=1.0, scale=1.0)
                    nc.vector.tensor_scalar(out=ct[:, 1:w + 1], in0=ct[:, 1:w + 1],
                                            scalar1=-1.0, scalar2=None, op0=OP.add)
                if m3_mode == "act":
                    nc.scalar.activation(qt[:, 1:w + 1], qt[:, 1:w + 1], ACT.Copy,
                                         bias=R3 - 1.0, scale=1.0)
                nc.gpsimd.memset(j["at"][:, 0:1], 0.0)
                # zero the t-shift pad the y-scan reads (guards NaN garbage)
                nc.gpsimd.memset(j["tt"][:, w + 1:w + 2], 0.0)
                return j

            def st_prep(j):
                """m3 = Q+(r-1) [DVE TS] and g = A2[k-1]*C."""
                w, a2, ct, qt = j["w"], j["a2"], j["ct"], j["qt"]
                if m3_mode != "act":
                    nc.vector.tensor_scalar(out=qt[:, 1:w + 1], in0=qt[:, 1:w + 1],
                                            scalar1=R3 - 1.0, scalar2=None,
                                            op0=OP.add)
                eg = ("dve" if (j["jidx"] < 2 or j["jidx"] >= j["njobs"] - 4)
                      else pick(eng_g, j["jidx"], j["njobs"]))
                product(eg, j["gt"][:, 1:w + 1],
                        a2[:, 0:w], ct[:, 1:w + 1])

            def is_edge(j):
                return (j["jidx"] < lat_edge[0]
                        or j["jidx"] >= j["njobs"] - lat_edge[1])


            def st_nr(j):
                """nr = m3 + g*m3[-1] (2t) or forward scan."""
                w = j["w"]
                if nr_mode == "sub":
                    nc.vector.tensor_tensor(out=j["nr"][:, 0:w],
                                            in0=j["qt"][:, 1:w + 1],
                                            in1=j["gt"][:, 1:w + 1],
                                            op=OP.subtract)
                elif nr_mode == "2t" and not is_edge(j):
                    nc.vector.tensor_tensor(out=j["nr"][:, 0:w],
                                            in0=j["gt"][:, 1:w + 1],
                                            in1=j["qt"][:, 0:w], op=OP.mult)
                    nc.gpsimd.dma_start(out=j["nr"][:, 0:w],
                                        in_=j["qt"][:, 1:w + 1], accum_op=OP.add)
                else:
                    nc.vector.tensor_tensor_scan(
                        out=j["nr"][:, 0:w], data0=j["gt"][:, 1:w + 1],
                        data1=j["qt"][:, 1:w + 1],
                        initial=0.0, op0=OP.mult, op1=OP.add,
                    )

            def st_q(j):
                """q = A2*nr into gt (g dead); t = C*nr into tt."""
                w = j["w"]
                product("dve", j["gt"][:, 1:w + 1],
                        j["a2"][:, 1:w + 1], j["nr"][:, 0:w])
                product(pick(eng_t, j["jidx"], j["njobs"]), j["tt"][:, 1:w + 1],
                        j["ct"][:, 1:w + 1], j["nr"][:, 0:w])

            def st_w(j):
                """w = f + (q*f)[-1] (2t) into at (alpha dead), or scan."""
                w = j["w"]
                dom_lo = j["s"] - j["oo"]
                fbs = fb[:, dom_lo:dom_lo + w]
                nc.vector.tensor_tensor(out=j["at"][:, 1:w + 1],
                                        in0=j["gt"][:, 1:w + 1],
                                        in1=fbs, op=OP.mult)
                if not is_edge(j):
                    nc.gpsimd.dma_start(out=j["at"][:, 0:w], in_=fbs,
                                        accum_op=OP.add)
                else:
                    nc.vector.tensor_tensor(out=j["at"][:, 0:w],
                                            in0=j["at"][:, 0:w], in1=fbs,
                                            op=OP.add)

            def st_y(j):
                """backward scan: y_i = t_{i+1}*y_{i+1} - w_i, into qt."""
                w = j["w"]
                nc.vector.tensor_tensor_scan(
                    out=j["qt"][:, 0:w][:, ::-1],
                    data0=j["tt"][:, 2:w + 2][:, ::-1],
                    data1=j["at"][:, 0:w][:, ::-1],
                    initial=0.0, op0=OP.mult, op1=OP.subtract,
                )

            def st_u(j):
                """u = nr*y into ct (C dead), DMA out."""
                oo, s, r0, m = j["oo"], j["s"], j["r0"], j["ssz"]
                ut = j["ct"]
                eng = "dve" if is_edge(j) else pick(eng_u, j["jidx"], j["njobs"])
                product(eng, ut[:, 0:m],
                        j["nr"][:, oo:oo + m], j["qt"][:, oo:oo + m])
                nc.sync.dma_start(out=out_d[r0:r0 + PB, s:s + m], in_=ut[:, 0:m])

            stages = [st_prep, st_nr, st_q, st_w, st_y, st_u]
            with tc.tile_pool(name="jobs", bufs=bufs) as pool:
                live = []
                nj = len(jobs)
                pieces = list(fb_pieces)
                for k in range(nj + max(lags)):
                    if k < nj:
                        r0, s, ssz = jobs[k]
                        live.append(front(pool, r0, s, ssz, k, nj))
                    if pieces and k >= 1:
                        lo, hi = pieces.pop(0)
                        nc.sync.dma_start(out=fb[:, lo:hi], in_=fb_d[:, lo:hi])
                    for fn, lag in zip(stages, lags):
                        i = k - lag
                        if 0 <= i < nj:
                            fn(live[i])
    return nc


_cached = None


def _get_program():
    global _cached
    if _cached is None:
        nc = bacc.Bacc("TRN2", target_bir_lowering=False, debug=False)
        build_core_program(nc)
        nc.compile()
        _cached = nc
    return _cached


def _in_maps(alpha, f):
    alpha16 = np.ascontiguousarray(alpha.astype(bfloat16))
    fb = np.ascontiguousarray(
        np.broadcast_to(f.astype(bfloat16).reshape(1, N), (PB, N))
    )
    return [
        {"alpha": alpha16[c * RPC:(c + 1) * RPC], "fb": fb}
        for c in range(NCORES)
    ]


def kernel(alpha: np.ndarray, f: np.ndarray) -> np.ndarray:
    alpha = np.ascontiguousarray(alpha, dtype=np.float32)
    f = np.ascontiguousarray(f, dtype=np.float32)
    nc = _get_program()
    res = bass_utils.run_bass_kernel_spmd(nc, _in_maps(alpha, f),
                                          core_ids=list(range(NCORES)))
    out = np.concatenate([r["out"] for r in res.results], axis=0)
    return out.astype(np.float32)


if __name__ == "__main__":
    rng = np.random.default_rng(0)
    a = (0.3 * rng.random((B, N))).astype(np.float32)
    fv = rng.standard_normal(N).astype(np.float32)
    u = kernel(a, fv)
    print(u.shape, u.dtype, np.abs(u).max())
